# revision 12
# baseline (speedup 1.0000x reference)
"""Multi-head attention TRN2 kernel (B=2, N=2048, D=1024, H=16).

Sharding: tensor-parallel over heads. Each of the 8 cores owns 2 heads
(both batch elements) end-to-end through QKV projection and attention,
then the per-head attention outputs are AllGathered (bf16, per batch
half) and each core computes a 128-column slice of the output
projection.

QKV matmuls run in float32r (full PE rate for >=256-wide outputs);
q/k/v are rounded to bf16 for the attention matmuls, and the projection
path (AllGather payload, Wproj, projection matmul) is bf16 as well
(~3e-3 scaled error, well under the 2e-2 gate).

Softmax runs without max-subtraction (scores are O(5); exp is safe in
fp32): S^T is computed key-major via matmul(lhsT=kT, rhs=qT), exp'd on
ScalarE, and the denominator comes from a ones-column appended to V in
the P^T@V matmul. The S->exp->PV software pipeline is two steps deep so
ScalarE's exp latency stays off the PE critical path. PSUM is released
with a single [65 x IBLK] copy; the reciprocal-broadcast-normalize
chain runs off the critical path (approx reciprocal on DVE, DRAM
broadcast round-trip on the sync queue, multiply on GpSimd).

Emission order interleaves phases (QKV b0 -> attention b0/ib0 -> QKV b1
-> remaining attention -> projection) so ScalarE's exp stream starts
early and input DMA overlaps attention. The ScalarE queue carries ONLY
the exps; all other DMA triggers live on sync/gpsimd so nothing blocks
the softmax pipeline.

All host-side tensors are laid out so every DMA descriptor is >=2KB
contiguous per partition.

Self-contained: hardcodes shapes from the problem spec.
"""

import sys

for _p in ("/opt/trn_rl_repo", "/root/.axon_site/_ro/trn_rl_repo"):
    if _p not in sys.path:
        sys.path.append(_p)

import numpy as np
from contextlib import ExitStack

import concourse.bass as bass
import concourse.tile as tile
from concourse import mybir, bacc
from concourse.bass_utils import run_bass_kernel_spmd

F32 = mybir.dt.float32
F32R = mybir.dt.float32r
BF16 = mybir.dt.bfloat16
EXP = mybir.ActivationFunctionType.Exp

B = 2
N = 2048
D = 1024
H = 16
DEPTH = 64
TOK = B * N            # 4096 tokens total (both batches)
KC = D // 128          # 8 contraction chunks of 128
NBLK = TOK // 512      # 8 token blocks for streaming projections
SCALE = 1.0 / np.sqrt(DEPTH)
NCORES = 8
IBLK = 1024            # query-block width in attention
NSUB = IBLK // 512     # matmuls per psum tile (N<=512 for 4-byte dtypes)
HALF = N // 2


def build_nc(qkv_dt=F32R, attn_dt=F32R, proj_dt=BF16):
    nc = bacc.Bacc(None)

    def dram_dt(dt):
        return F32 if dt == F32R else dt

    def cast(ap, dt):
        return ap.bitcast(F32R) if dt == F32R else ap

    # x^T stored block-major: [128, blk, kc, 512] so each 512-token block
    # is 16KB contiguous per partition.
    xt = nc.dram_tensor("xt", [128, NBLK, KC, 512], dram_dt(qkv_dt),
                        kind="ExternalInput")
    # weights pre-permuted to [128, kc, 128] (4KB contiguous/partition)
    wq = nc.dram_tensor("wq", [128, KC, 128], dram_dt(qkv_dt), kind="ExternalInput")
    wk = nc.dram_tensor("wk", [128, KC, 128], dram_dt(qkv_dt), kind="ExternalInput")
    wv = nc.dram_tensor("wv", [128, KC, 128], dram_dt(qkv_dt), kind="ExternalInput")
    wp = nc.dram_tensor("wp", [128, KC, 128], dram_dt(proj_dt), kind="ExternalInput")
    bq = nc.dram_tensor("bq", [128, 1], F32, kind="ExternalInput")
    bk = nc.dram_tensor("bk", [128, 1], F32, kind="ExternalInput")
    bv = nc.dram_tensor("bv", [128, 1], F32, kind="ExternalInput")
    bp = nc.dram_tensor("bp", [128, 1], F32, kind="ExternalInput")
    ident = nc.dram_tensor(
        "ident", [128, 128], dram_dt(attn_dt), kind="ExternalInput"
    )
    ones = nc.dram_tensor("ones", [128, 1], dram_dt(attn_dt), kind="ExternalInput")
    out = nc.dram_tensor("o", [128, TOK], F32, kind="ExternalOutput")

    # Collective staging (split per (batch, half) so early AllGathers
    # overlap later attention).
    ag_in = [
        [nc.dram_tensor(f"ag_in{b}_{h}", [128, HALF], proj_dt) for h in range(2)]
        for b in range(B)
    ]
    ag_out = [
        [
            nc.dram_tensor(f"ag_out{b}_{h}", [D, HALF], proj_dt,
                           addr_space="Shared")
            for h in range(2)
        ]
        for b in range(B)
    ]

    with tile.TileContext(nc) as tc, ExitStack() as ctx:
        wpool = ctx.enter_context(tc.tile_pool(name="w", bufs=1))
        qkpool = ctx.enter_context(tc.tile_pool(name="qk", bufs=1))
        vpool = ctx.enter_context(tc.tile_pool(name="v2", bufs=1))
        xpool = ctx.enter_context(tc.tile_pool(name="x", bufs=4))
        vtpool = ctx.enter_context(tc.tile_pool(name="vt", bufs=2))
        ptpool = ctx.enter_context(tc.tile_pool(name="pt", bufs=3))
        unpool = ctx.enter_context(tc.tile_pool(name="un", bufs=2))
        rpool = ctx.enter_context(tc.tile_pool(name="r", bufs=2))
        rdpool = ctx.enter_context(tc.tile_pool(name="rd", bufs=2, space="DRAM"))
        apool = ctx.enter_context(tc.tile_pool(name="ap", bufs=2))
        oupool = ctx.enter_context(tc.tile_pool(name="ou", bufs=2))
        # PSUM budget (8 banks of 2KB/partition):
        #   ps_one (bufs=1): po [65,1024]                -> 2 banks
        #   ps_mm (bufs=2): mmA [128,512] qkv/transpose/proj -> 2 banks
        #   ps_two (bufs=2): ss [128,1024]               -> 4 banks
        ps_one = ctx.enter_context(tc.tile_pool(name="ps1", bufs=1, space="PSUM"))
        ps_mm = ctx.enter_context(tc.tile_pool(name="psm", bufs=2, space="PSUM"))
        ps_two = ctx.enter_context(tc.tile_pool(name="ps2", bufs=2, space="PSUM"))

        # ---- weights / constants ----
        w_q = wpool.tile([128, KC, 128], qkv_dt, tag="w_q")
        w_k = wpool.tile([128, KC, 128], qkv_dt, tag="w_k")
        w_v = wpool.tile([128, KC, 128], qkv_dt, tag="w_v")
        w_p = wpool.tile([128, KC, 128], proj_dt, tag="w_p")
        # q weights first on the sync queue so the first matmul can start
        # as soon as xb0 lands; the rest stream in behind on gpsimd.
        nc.sync.dma_start(out=w_q, in_=cast(wq[:], qkv_dt))
        nc.sync.dma_start(out=w_k, in_=cast(wk[:], qkv_dt))
        nc.sync.dma_start(out=w_v, in_=cast(wv[:], qkv_dt))
        nc.gpsimd.dma_start(out=w_p, in_=cast(wp[:], proj_dt))
        b_q = wpool.tile([128, 1], F32, tag="b_q")
        b_k = wpool.tile([128, 1], F32, tag="b_k")
        b_v = wpool.tile([128, 1], F32, tag="b_v")
        b_p = wpool.tile([128, 1], F32, tag="b_p")
        for t, src in ((b_q, bq), (b_k, bk), (b_v, bv), (b_p, bp)):
            nc.gpsimd.dma_start(out=t, in_=src[:])
        id_t = wpool.tile([128, 128], attn_dt, tag="id_t")
        nc.gpsimd.dma_start(out=id_t, in_=cast(ident[:], attn_dt))

        # qT/kT: [feature 128 (= 2 heads x 64), token 4096]; head hl in rows
        # hl*64:(hl+1)*64 so both S^T operands share a partition base.
        qT = qkpool.tile([128, TOK], attn_dt, tag="qT")
        kT = qkpool.tile([128, TOK], attn_dt, tag="kT")
        # V2: [token part, 32 token-chunks, 130]: v_h0 | ones | v_h1 | ones
        V2 = vpool.tile([128, TOK // 128, 130], attn_dt, tag="V2")
        nc.scalar.dma_start(
            out=V2[:, :, 64:65],
            in_=cast(ones[:].to_broadcast((128, TOK // 128, 1)), attn_dt),
        )
        nc.scalar.dma_start(
            out=V2[:, :, 129:130],
            in_=cast(ones[:].to_broadcast((128, TOK // 128, 1)), attn_dt),
        )

        # ================= phase A: QKV projection for a block range =====
        def qkv_blocks(blo, bhi):
            xbs = []
            for blk in range(blo, bhi):
                xb = xpool.tile([128, KC, 512], qkv_dt, tag="xb")
                eng = nc.sync if blk % 2 == 0 else nc.gpsimd
                eng.dma_start(out=xb, in_=cast(xt[:, blk], qkv_dt))
                xbs.append(xb)
            for blk, xb in zip(range(blo, bhi), xbs):
                for name, w_t, b_t in (
                    ("q", w_q, b_q), ("k", w_k, b_k), ("v", w_v, b_v)
                ):
                    t0 = blk * 512
                    ps = ps_mm.tile([128, 512], F32, tag="mmA")
                    for kc in range(KC):
                        nc.tensor.matmul(
                            out=ps,
                            lhsT=w_t[:, kc, :],
                            rhs=xb[:, kc, :],
                            start=(kc == 0),
                            stop=(kc == KC - 1),
                        )
                    if name == "q":
                        nc.vector.tensor_scalar_add(
                            out=qT[:, t0 : t0 + 512], in0=ps, scalar1=b_t
                        )
                    elif name == "k":
                        nc.vector.tensor_scalar_add(
                            out=kT[:, t0 : t0 + 512], in0=ps, scalar1=b_t
                        )
                    else:
                        vtmp = vtpool.tile([128, 512], attn_dt, tag="vtmp")
                        nc.vector.tensor_scalar_add(out=vtmp, in0=ps, scalar1=b_t)
                        # transpose 4x [128,128] -> V2 token chunks
                        for s in range(4):
                            ch = blk * 4 + s
                            ps_t = ps_mm.tile([128, 128], attn_dt, tag="mmA")
                            nc.tensor.transpose(
                                out=ps_t,
                                in_=vtmp[:, s * 128 : (s + 1) * 128],
                                identity=id_t,
                            )
                            nc.vector.tensor_copy(
                                out=V2[:, ch, 0:64], in_=ps_t[:, 0:64]
                            )
                            nc.vector.tensor_copy(
                                out=V2[:, ch, 65:129], in_=ps_t[:, 64:128]
                            )

        # ===== phase B: attention for one (batch, query-block) ==========
        def attn_chunk(b, ib):
            for hl in range(2):
                hs = hl * 64
                voff = hl * 65
                i0 = b * N + ib * IBLK
                ps_o = ps_one.tile([65, IBLK], F32, tag="po")
                NJC = N // 128

                def s_step(jc):
                    j0 = b * N + jc * 128
                    ps_s = ps_two.tile([128, IBLK], F32, tag="ss")
                    for su in range(NSUB):
                        nc.tensor.matmul(
                            out=ps_s[:, su * 512 : (su + 1) * 512],
                            lhsT=kT[hs : hs + 64, j0 : j0 + 128],
                            rhs=qT[
                                hs : hs + 64,
                                i0 + su * 512 : i0 + (su + 1) * 512,
                            ],
                            start=True,
                            stop=True,
                        )
                    pt = ptpool.tile([128, IBLK], attn_dt, tag="pt")
                    nc.scalar.activation(
                        out=pt, in_=ps_s, func=EXP, scale=float(SCALE)
                    )
                    return pt

                # two-deep software pipeline: exp(jc+1) and exp(jc+2) are
                # in flight while PV(jc) streams, so ScalarE latency never
                # stalls the PE.
                pt_q = [s_step(0), s_step(1)]
                for jc in range(NJC):
                    pt_cur = pt_q.pop(0)
                    if jc + 2 < NJC:
                        pt_q.append(s_step(jc + 2))
                    for su in range(NSUB):
                        nc.tensor.matmul(
                            out=ps_o[:, su * 512 : (su + 1) * 512],
                            lhsT=V2[:, ((b * N + jc * 128) // 128), voff : voff + 65],
                            rhs=pt_cur[:, su * 512 : (su + 1) * 512],
                            start=(jc == 0),
                            stop=(jc == NJC - 1),
                        )
                # single copy (rows + denominator) releases PSUM; then
                # recip -> DRAM broadcast -> normalize, all off the PE path.
                und = unpool.tile([65, IBLK], F32, tag="und")
                nc.vector.tensor_copy(out=und, in_=ps_o)
                rd = rdpool.tile([1, IBLK], F32, tag="rd")
                nc.sync.dma_start(out=rd, in_=und[64:65, :])
                rr = rpool.tile([64, IBLK], F32, tag="rr")
                nc.sync.dma_start(out=rr, in_=rd.to_broadcast((64, IBLK)))
                rcp = rpool.tile([64, IBLK], F32, tag="rcp")
                nc.vector.reciprocal_approx_fast(out=rcp, in_=rr)
                unr = unpool.tile([64, IBLK], proj_dt, tag="unr")
                nc.gpsimd.tensor_mul(out=unr, in0=und[0:64, :], in1=rcp)
                nc.sync.dma_start(out=ag_in[b][ib][hs : hs + 64, :], in_=unr)
            # both heads of (b, ib-half) staged: gather it
            nc.gpsimd.collective_compute(
                "AllGather",
                mybir.AluOpType.bypass,
                ins=[ag_in[b][ib][:]],
                outs=[ag_out[b][ib][:]],
                replica_groups=[list(range(NCORES))],
            )

        # ======= phase D: output projection (128 columns/core) =======
        def proj_half(b, hf):
            ag_r = ag_out[b][hf].rearrange("(kc p) t -> p kc t", p=128)
            ab = apool.tile([128, KC, HALF], proj_dt, tag="ab")
            nc.gpsimd.dma_start(out=ab, in_=ag_r)
            for i2 in range(HALF // 512):
                i0 = i2 * 512
                ps = ps_mm.tile([128, 512], F32, tag="mmA")
                for kc in range(KC):
                    nc.tensor.matmul(
                        out=ps,
                        lhsT=w_p[:, kc, :],
                        rhs=ab[:, kc, i0 : i0 + 512],
                        start=(kc == 0),
                        stop=(kc == KC - 1),
                    )
                ot = oupool.tile([128, 512], F32, tag="ot")
                nc.vector.tensor_scalar_add(out=ot, in0=ps, scalar1=b_p)
                to = b * N + hf * HALF + i0
                nc.sync.dma_start(out=out[:, to : to + 512], in_=ot)

        # ---- interleaved emission ----
        qkv_blocks(0, 4)       # b0 tokens
        attn_chunk(0, 0)       # attention b0 first half + AG
        qkv_blocks(4, 8)       # b1 tokens (DMA overlaps attention above)
        attn_chunk(0, 1)
        attn_chunk(1, 0)
        attn_chunk(1, 1)
        proj_half(0, 0)
        proj_half(0, 1)
        proj_half(1, 0)
        proj_half(1, 1)

    nc.compile()
    return nc


def np_dt(dt):
    return mybir.dt.np(F32 if dt == F32R else dt)


def prep_in_maps(x, Wqkv, bqkv, Wproj, bproj, qkv_dt=F32R, attn_dt=F32R,
                 proj_dt=BF16):
    x = np.asarray(x, dtype=np.float32)
    Wqkv = np.asarray(Wqkv, dtype=np.float32)
    bqkv = np.asarray(bqkv, dtype=np.float32)
    Wproj = np.asarray(Wproj, dtype=np.float32)
    bproj = np.asarray(bproj, dtype=np.float32)

    # x^T block-major: [128, blk, kc, 512]; row kc*128+p of x^T.
    xT = np.ascontiguousarray(x.reshape(TOK, D).T)     # [D, TOK]
    xtn = np.ascontiguousarray(
        xT.reshape(KC, 128, NBLK, 512).transpose(1, 2, 0, 3)
    ).astype(np_dt(qkv_dt))
    identity = np.eye(128, dtype=np_dt(attn_dt))
    ones_col = np.ones((128, 1), dtype=np_dt(attn_dt))

    def perm_w(w):  # [D, 128] -> [128, KC, 128] with row kc*128+p
        return np.ascontiguousarray(w.reshape(KC, 128, -1).transpose(1, 0, 2))

    # AllGather output rows are rank-major: row c*128 + hl*64 + d holds
    # feature (2c+hl)*64 + d; permute Wproj's contraction rows to match.
    wp_row_perm = np.empty(D, dtype=np.int64)
    for cc in range(NCORES):
        for hlhl in range(2):
            rows = np.arange(64)
            wp_row_perm[cc * 128 + hlhl * 64 + rows] = (2 * cc + hlhl) * 64 + rows

    # qkv column index for (head h, depth d, which): h*192 + d*3 + which
    d_idx = np.arange(DEPTH)
    in_maps = []
    for c in range(NCORES):
        h0, h1 = 2 * c, 2 * c + 1
        qcols = np.concatenate([h0 * 192 + 3 * d_idx, h1 * 192 + 3 * d_idx])
        kcols = qcols + 1
        vcols = qcols + 2
        in_maps.append(
            {
                "xt": xtn,
                "wq": perm_w(Wqkv[:, qcols]).astype(np_dt(qkv_dt)),
                "wk": perm_w(Wqkv[:, kcols]).astype(np_dt(qkv_dt)),
                "wv": perm_w(Wqkv[:, vcols]).astype(np_dt(qkv_dt)),
                "wp": perm_w(
                    Wproj[wp_row_perm, 128 * c : 128 * (c + 1)]
                ).astype(np_dt(proj_dt)),
                "bq": np.ascontiguousarray(bqkv[qcols]).reshape(128, 1),
                "bk": np.ascontiguousarray(bqkv[kcols]).reshape(128, 1),
                "bv": np.ascontiguousarray(bqkv[vcols]).reshape(128, 1),
                "bp": np.ascontiguousarray(
                    bproj[128 * c : 128 * (c + 1)]
                ).reshape(128, 1),
                "ident": identity,
                "ones": ones_col,
            }
        )
    return in_maps


def assemble(results):
    outT = np.concatenate([r["o"] for r in results], axis=0)  # [D, TOK]
    return np.ascontiguousarray(outT.T).reshape(B, N, D).astype(np.float32)


CONFIG = {"qkv_dt": F32R, "attn_dt": F32R, "proj_dt": BF16}

_NC_CACHE = {}


def get_nc():
    if "nc" not in _NC_CACHE:
        _NC_CACHE["nc"] = build_nc(**CONFIG)
    return _NC_CACHE["nc"]


def kernel(x, Wqkv, bqkv, Wproj, bproj):
    nc = get_nc()
    in_maps = prep_in_maps(x, Wqkv, bqkv, Wproj, bproj, **CONFIG)
    res = run_bass_kernel_spmd(nc, in_maps, list(range(NCORES)))
    return assemble(res.results)


# revision 28
# speedup vs baseline: 1.7935x; 1.7935x over previous
"""Multi-head attention TRN2 kernel (B=2, N=2048, D=1024, H=16).

Sharding: tensor-parallel over heads. Each of the 8 cores owns 2 heads
(both batch elements) end-to-end through QKV projection and attention,
then the per-head attention outputs are AllGathered (bf16, per batch
half) and each core computes a 128-column slice of the output
projection.

Dtypes: x and the QKV weights are bf16 (halves the input streaming);
the attention matmuls run in float32r (same 1 cyc/row PE rate as bf16
for >=256-wide outputs, and the bf16 S/PV/transpose path miscompiles on
HW); the projection path (AllGather payload, Wproj, projection matmul)
is bf16. Net ~4e-3 scaled error, 5x under the 2e-2 gate.

Softmax runs without max-subtraction (scores are O(5); exp is safe in
fp32): S^T is computed key-major via matmul(lhsT=kT, rhs=qT), exp'd on
ScalarE, and the denominator comes from a ones-column appended to V in
the P^T@V matmul. The S->exp->PV software pipeline is two steps deep.
Normalization happens off the PE path: one [65,1024] DVE copy releases
PSUM, then approx-reciprocal + a DRAM broadcast round-trip + multiply
(all on DVE/GpSimd/sync queues) feed the AllGather staging.

The attention phase is ScalarE-paced (exp of a [128,1024] chunk takes
~1.1us vs ~0.93us of PE work per step), and on TRN2 every PE stall
resets the tensor engine's DVFS ramp, halving its clock. So all other
matmul work is split into ~2-matmul micro-units that attention steps
pull BETWEEN their S and PV matmuls (the exact spot where the PE would
otherwise stall on exp): batch-1 QKV chains stream through batch-0's
attention, the batch-0 projection streams through batch-1's attention,
and input DMA + AllGathers overlap attention compute throughout.

The last attention chunk uses a low-latency staging variant: the
reciprocal row is replicated across partitions with a rank-1 matmul
(no DRAM round trip, ~8us from last PV to the AllGather trigger), and
its AllGather is split per head group so the first head's gather flies
while the second head is still computing. The final projection
contracts 4 kc chunks from each per-head gathered half (w_p2a/w_p2b
carry the matching row permutations).

All host-side tensors are laid out so every DMA descriptor is >=2KB
contiguous per partition (strided layouts ran the HBM queues at
~75GB/s; these run near peak).

Self-contained: hardcodes shapes from the problem spec.
"""

import sys

for _p in ("/opt/trn_rl_repo", "/root/.axon_site/_ro/trn_rl_repo"):
    if _p not in sys.path:
        sys.path.append(_p)

import numpy as np
from contextlib import ExitStack

import concourse.bass as bass
import concourse.tile as tile
from concourse import mybir, bacc
from concourse.bass_utils import run_bass_kernel_spmd

F32 = mybir.dt.float32
F32R = mybir.dt.float32r
BF16 = mybir.dt.bfloat16
EXP = mybir.ActivationFunctionType.Exp

B = 2
N = 2048
D = 1024
H = 16
DEPTH = 64
TOK = B * N            # 4096 tokens total (both batches)
KC = D // 128          # 8 contraction chunks of 128
NBLK = TOK // 512      # 8 token blocks for streaming projections
SCALE = 1.0 / np.sqrt(DEPTH)
NCORES = 8
IBLK = 1024            # query-block width in attention
NSUB = IBLK // 512     # matmuls per psum tile (N<=512 for 4-byte dtypes)
HALF = N // 2
NJC = N // 128         # key chunks per batch


def build_nc(qkv_dt=F32R, attn_dt=F32R, proj_dt=BF16):
    nc = bacc.Bacc(None)

    def dram_dt(dt):
        return F32 if dt == F32R else dt

    def cast(ap, dt):
        return ap.bitcast(F32R) if dt == F32R else ap

    # x^T stored block-major: [128, blk, kc, 512] so each 512-token block
    # is 16KB contiguous per partition.
    xt = nc.dram_tensor("xt", [128, NBLK, KC, 512], dram_dt(qkv_dt),
                        kind="ExternalInput")
    # weights pre-permuted to [128, kc, 128] (4KB contiguous/partition)
    wq = nc.dram_tensor("wq", [128, KC, 128], dram_dt(qkv_dt), kind="ExternalInput")
    wk = nc.dram_tensor("wk", [128, KC, 128], dram_dt(qkv_dt), kind="ExternalInput")
    wv = nc.dram_tensor("wv", [128, KC, 128], dram_dt(qkv_dt), kind="ExternalInput")
    wp = nc.dram_tensor("wp", [128, KC, 128], dram_dt(proj_dt), kind="ExternalInput")
    bq = nc.dram_tensor("bq", [128, 1], F32, kind="ExternalInput")
    bk = nc.dram_tensor("bk", [128, 1], F32, kind="ExternalInput")
    bv = nc.dram_tensor("bv", [128, 1], F32, kind="ExternalInput")
    bp = nc.dram_tensor("bp", [128, 1], F32, kind="ExternalInput")
    ident = nc.dram_tensor(
        "ident", [128, 128], dram_dt(attn_dt), kind="ExternalInput"
    )
    ones = nc.dram_tensor("ones", [128, 1], dram_dt(attn_dt), kind="ExternalInput")
    out = nc.dram_tensor("o", [128, TOK], F32, kind="ExternalOutput")

    # Collective staging (split per (batch, half) so early AllGathers
    # overlap later attention).
    ag_in = [
        [nc.dram_tensor(f"ag_in{b}_{h}", [128, HALF], proj_dt) for h in range(2)]
        for b in range(B)
    ]
    ag_out = [
        [
            nc.dram_tensor(f"ag_out{b}_{h}", [D, HALF], proj_dt,
                           addr_space="Shared")
            for h in range(2)
        ]
        for b in range(B)
    ]
    # split staging for the LAST chunk: one gather per head group, so the
    # first head's AllGather overlaps the second head's attention.
    ag_in3 = [nc.dram_tensor(f"ag_in3_{h}", [64, HALF], proj_dt)
              for h in range(2)]
    ag_out3 = [nc.dram_tensor(f"ag_out3_{h}", [D // 2, HALF], proj_dt,
                              addr_space="Shared") for h in range(2)]
    wp2a = nc.dram_tensor("wp2a", [128, KC // 2, 128], dram_dt(proj_dt),
                          kind="ExternalInput")
    wp2b = nc.dram_tensor("wp2b", [128, KC // 2, 128], dram_dt(proj_dt),
                          kind="ExternalInput")

    with tile.TileContext(nc) as tc, ExitStack() as ctx:
        wpool = ctx.enter_context(tc.tile_pool(name="w", bufs=1))
        qkpool = ctx.enter_context(tc.tile_pool(name="qk", bufs=1))
        vpool = ctx.enter_context(tc.tile_pool(name="v2", bufs=1))
        xpool = ctx.enter_context(tc.tile_pool(name="x", bufs=4))
        vtpool = ctx.enter_context(tc.tile_pool(name="vt", bufs=2))
        ptpool = ctx.enter_context(tc.tile_pool(name="pt", bufs=3))
        unpool = ctx.enter_context(tc.tile_pool(name="un", bufs=2))
        rpool = ctx.enter_context(tc.tile_pool(name="r", bufs=1))
        rdpool = ctx.enter_context(tc.tile_pool(name="rd", bufs=2, space="DRAM"))
        apool = ctx.enter_context(tc.tile_pool(name="ap", bufs=2))
        oupool = ctx.enter_context(tc.tile_pool(name="ou", bufs=2))
        # PSUM budget (8 banks of 2KB/partition):
        #   ps_one (bufs=1): po [65,1024]                -> 2 banks
        #   ps_mm (bufs=2): mmA [128,512] qkv/transpose/proj -> 2 banks
        #   ps_two (bufs=2): ss [128,1024]               -> 4 banks
        ps_one = ctx.enter_context(tc.tile_pool(name="ps1", bufs=1, space="PSUM"))
        ps_mm = ctx.enter_context(tc.tile_pool(name="psm", bufs=2, space="PSUM"))
        ps_two = ctx.enter_context(tc.tile_pool(name="ps2", bufs=2, space="PSUM"))

        # ---- weights / constants ----
        w_q = wpool.tile([128, KC, 128], qkv_dt, tag="w_q")
        w_k = wpool.tile([128, KC, 128], qkv_dt, tag="w_k")
        w_v = wpool.tile([128, KC, 128], qkv_dt, tag="w_v")
        w_p = wpool.tile([128, KC, 128], proj_dt, tag="w_p")
        w_p2a = wpool.tile([128, KC // 2, 128], proj_dt, tag="w_p2a")
        w_p2b = wpool.tile([128, KC // 2, 128], proj_dt, tag="w_p2b")
        b_q = wpool.tile([128, 1], F32, tag="b_q")
        b_k = wpool.tile([128, 1], F32, tag="b_k")
        b_v = wpool.tile([128, 1], F32, tag="b_v")
        b_p = wpool.tile([128, 1], F32, tag="b_p")
        id_t = wpool.tile([128, 128], attn_dt, tag="id_t")

        # qT/kT: [feature 128 (= 2 heads x 64), token 4096]; head hl in rows
        # hl*64:(hl+1)*64 so both S^T operands share a partition base.
        qT = qkpool.tile([128, TOK], attn_dt, tag="qT")
        kT = qkpool.tile([128, TOK], attn_dt, tag="kT")
        # V2: [token part, 32 token-chunks, 130]: v_h0 | ones | v_h1 | ones
        V2 = vpool.tile([128, TOK // 128, 130], attn_dt, tag="V2")

        # ---- startup DMA order: xb0 (gpsimd) + w_q (sync) in parallel
        # so the first QKV chain can start at ~20us.
        xbs = {}

        def issue_xb(blk):
            xb = xpool.tile([128, KC, 512], qkv_dt, tag="xb")
            eng = nc.sync if blk % 2 == 0 else nc.gpsimd
            eng.dma_start(out=xb, in_=cast(xt[:, blk], qkv_dt))
            xbs[blk] = xb

        nc.gpsimd.dma_start(out=b_q, in_=bq[:])
        xb0t = xpool.tile([128, KC, 512], qkv_dt, tag="xb")
        nc.gpsimd.dma_start(out=xb0t[:, 0:4], in_=cast(xt[:, 0, 0:4], qkv_dt))
        nc.sync.dma_start(out=w_q, in_=cast(wq[:], qkv_dt))
        nc.sync.dma_start(out=xb0t[:, 4:8], in_=cast(xt[:, 0, 4:8], qkv_dt))
        xbs[0] = xb0t
        nc.sync.dma_start(out=w_k, in_=cast(wk[:], qkv_dt))
        nc.sync.dma_start(out=w_v, in_=cast(wv[:], qkv_dt))
        for t, src in ((b_k, bk), (b_v, bv)):
            nc.gpsimd.dma_start(out=t, in_=src[:])
        nc.gpsimd.dma_start(out=id_t, in_=cast(ident[:], attn_dt))
        issue_xb(1)
        issue_xb(2)
        issue_xb(3)
        nc.gpsimd.dma_start(out=w_p, in_=cast(wp[:], proj_dt))
        nc.gpsimd.dma_start(out=w_p2a, in_=cast(wp2a[:], proj_dt))
        nc.gpsimd.dma_start(out=w_p2b, in_=cast(wp2b[:], proj_dt))
        nc.gpsimd.dma_start(out=b_p, in_=bp[:])
        # ones column of V2 via one tiny DMA + free-axis broadcast copies
        # (a broadcast DMA would generate 8192 4-byte descriptors).
        ones_r = wpool.tile([1, 128], attn_dt, tag="ones_r")
        nc.scalar.dma_start(
            out=ones_r, in_=cast(ones[:].rearrange("p o -> o p"), attn_dt)
        )
        nc.scalar.dma_start(
            out=V2[:, :, 64:65],
            in_=cast(ones[:].to_broadcast((128, TOK // 128, 1)), attn_dt),
        )
        nc.scalar.dma_start(
            out=V2[:, :, 129:130],
            in_=cast(ones[:].to_broadcast((128, TOK // 128, 1)), attn_dt),
        )

        # ---- work-unit emitters ----
        # Filler work (QKV chains, projection chains) is split into
        # micro-units of ~2 matmuls. Attention groups pull one unit
        # BETWEEN the S and PV matmuls of a step, so the PE has
        # independent work exactly where it would otherwise stall
        # waiting for ScalarE's exp — keeping the tensor engine
        # continuously busy (and at full DVFS clock).
        class Stream:
            def __init__(self):
                self.units = []

            def add(self, *units):
                self.units.extend(units)

            def pull(self):
                if self.units:
                    self.units.pop(0)()

            def drain(self):
                while self.units:
                    self.units.pop(0)()

        def qkv_chain_units(blk, name):
            """Micro-units for one q/k/v projection chain of a block."""
            w_t, b_t = {"q": (w_q, b_q), "k": (w_k, b_k), "v": (w_v, b_v)}[name]
            t0 = blk * 512
            state = {}

            def mk_mm(k0, k1):
                def f():
                    if "ps" not in state:
                        pst = ps_mm.tile([128, 512], F32, tag="mmA")
                        state["ps"] = pst
                    xb = xbs[blk]
                    for kc in range(k0, k1):
                        nc.tensor.matmul(
                            out=state["ps"], lhsT=w_t[:, kc, :],
                            rhs=xb[:, kc, :],
                            start=(kc == 0), stop=(kc == KC - 1),
                        )
                return f

            def fin():
                ps = state["ps"]
                if name == "q":
                    nc.vector.tensor_scalar_add(
                        out=qT[:, t0 : t0 + 512], in0=ps, scalar1=b_t
                    )
                elif name == "k":
                    nc.vector.tensor_scalar_add(
                        out=kT[:, t0 : t0 + 512], in0=ps, scalar1=b_t
                    )
                else:
                    vtmp = vtpool.tile([128, 512], attn_dt, tag="vtmp")
                    nc.vector.tensor_scalar_add(out=vtmp, in0=ps, scalar1=b_t)
                    state["vtmp"] = vtmp

            def mk_tr(sx):
                def f():
                    vtmp = state["vtmp"]
                    ch = blk * 4 + sx
                    ps_t = ps_mm.tile([128, 128], attn_dt, tag="mmA")
                    nc.tensor.transpose(
                        out=ps_t, in_=vtmp[:, sx * 128 : (sx + 1) * 128],
                        identity=id_t,
                    )
                    nc.vector.tensor_copy(out=V2[:, ch, 0:64], in_=ps_t[:, 0:64])
                    nc.vector.tensor_copy(
                        out=V2[:, ch, 65:129], in_=ps_t[:, 64:128]
                    )
                return f

            units = [mk_mm(0, 2), mk_mm(2, 4), mk_mm(4, 6), mk_mm(6, 8), fin]
            if name == "v":
                units += [mk_tr(sx) for sx in range(4)]
            return units

        def qkv_chain(blk, name):
            for u in qkv_chain_units(blk, name):
                u()

        abs_ = {}

        def issue_ab(b, hf):
            ag_r = ag_out[b][hf].rearrange("(kc p) t -> p kc t", p=128)
            ab = apool.tile([128, KC, HALF], proj_dt, tag="ab")
            nc.gpsimd.dma_start(out=ab, in_=ag_r)
            abs_[(b, hf)] = ab

        def proj_chain_units(b, hf, i2):
            i0 = i2 * 512
            state = {}

            def mk_mm(k0, k1):
                def f():
                    if "ps" not in state:
                        pst = ps_mm.tile([128, 512], F32, tag="mmA")
                        state["ps"] = pst
                    ab = abs_[(b, hf)]
                    for kc in range(k0, k1):
                        nc.tensor.matmul(
                            out=state["ps"], lhsT=w_p[:, kc, :],
                            rhs=ab[:, kc, i0 : i0 + 512],
                            start=(kc == 0), stop=(kc == KC - 1),
                        )
                return f

            def fin():
                ot = oupool.tile([128, 512], F32, tag="ot")
                nc.vector.tensor_scalar_add(out=ot, in0=state["ps"], scalar1=b_p)
                to = b * N + hf * HALF + i0
                nc.sync.dma_start(out=out[:, to : to + 512], in_=ot)

            return [mk_mm(0, 2), mk_mm(2, 4), mk_mm(4, 6), mk_mm(6, 8), fin]

        def proj_chain(b, hf, i2):
            for u in proj_chain_units(b, hf, i2):
                u()

        def attn_group(b, ib, hl, stream, pull_every=1, fast_stage=False):
            """One (batch, query-block, head) attention group. Pulls one
            stream unit between the S and PV matmuls of each step."""
            hs = hl * 64
            voff = hl * 65
            i0 = b * N + ib * IBLK
            ps_o = ps_one.tile([65, IBLK], F32, tag="po")

            def s_step(jc):
                j0 = b * N + jc * 128
                ps_s = ps_two.tile([128, IBLK], F32, tag="ss")
                for su in range(NSUB):
                    nc.tensor.matmul(
                        out=ps_s[:, su * 512 : (su + 1) * 512],
                        lhsT=kT[hs : hs + 64, j0 : j0 + 128],
                        rhs=qT[hs : hs + 64, i0 + su * 512 : i0 + (su + 1) * 512],
                        start=True, stop=True,
                    )
                pt = ptpool.tile([128, IBLK], attn_dt, tag="pt")
                nc.scalar.activation(out=pt, in_=ps_s, func=EXP, scale=float(SCALE))
                return pt

            pt_q = [s_step(0), s_step(1)]
            for jc in range(NJC):
                pt_cur = pt_q.pop(0)
                if jc + 2 < NJC:
                    pt_q.append(s_step(jc + 2))
                if jc % pull_every == 0:
                    stream.pull()
                for su in range(NSUB):
                    nc.tensor.matmul(
                        out=ps_o[:, su * 512 : (su + 1) * 512],
                        lhsT=V2[:, ((b * N + jc * 128) // 128), voff : voff + 65],
                        rhs=pt_cur[:, su * 512 : (su + 1) * 512],
                        start=(jc == 0), stop=(jc == NJC - 1),
                    )
            # single copy (rows + denominator) releases PSUM.
            und = unpool.tile([65, IBLK], F32, tag="und")
            nc.vector.tensor_copy(out=und, in_=ps_o)
            unr = unpool.tile([64, IBLK], proj_dt, tag="unr")
            if fast_stage:
                # recip of the whole tile (same DVE cost as one row), then
                # replicate the denominator row across 64 partitions with
                # a rank-1 matmul: ~8us from last PV to the AG trigger.
                rc65 = rpool.tile([65, IBLK], F32, tag="rc65")
                nc.vector.reciprocal_approx_fast(out=rc65, in_=und)
                rcr = rpool.tile([1, IBLK], attn_dt, tag="rcr")
                nc.vector.tensor_copy(out=rcr, in_=rc65[64:65, :])
                for su in range(NSUB):
                    ps_bc = ps_mm.tile([64, 512], F32, tag="mmA")
                    nc.tensor.matmul(
                        out=ps_bc,
                        lhsT=ones_r[:, 0:64],
                        rhs=rcr[:, su * 512 : (su + 1) * 512],
                        start=True, stop=True,
                    )
                    nc.vector.tensor_mul(
                        out=unr[:, su * 512 : (su + 1) * 512],
                        in0=und[0:64, su * 512 : (su + 1) * 512],
                        in1=ps_bc,
                    )
                nc.sync.dma_start(out=ag_in3[hl][:], in_=unr)
                nc.gpsimd.collective_compute(
                    "AllGather", mybir.AluOpType.bypass,
                    ins=[ag_in3[hl][:]], outs=[ag_out3[hl][:]],
                    replica_groups=[list(range(NCORES))],
                )
            else:
                rd = rdpool.tile([1, IBLK], F32, tag="rd")
                nc.gpsimd.dma_start(out=rd, in_=und[64:65, :])
                rr = rpool.tile([64, IBLK], F32, tag="rr")
                nc.gpsimd.dma_start(out=rr, in_=rd.to_broadcast((64, IBLK)))
                rcp = rpool.tile([64, IBLK], F32, tag="rcp")
                nc.vector.reciprocal_approx_fast(out=rcp, in_=rr)
                nc.gpsimd.tensor_mul(out=unr, in0=und[0:64, :], in1=rcp)
                nc.gpsimd.dma_start(out=ag_in[b][ib][hs : hs + 64, :], in_=unr)

        def attn_chunk(b, ib, stream, pull_every=1, fast_stage=False):
            attn_group(b, ib, 0, stream, pull_every, fast_stage)
            attn_group(b, ib, 1, stream, pull_every, fast_stage)
            if not fast_stage:
                nc.gpsimd.collective_compute(
                    "AllGather", mybir.AluOpType.bypass,
                    ins=[ag_in[b][ib][:]], outs=[ag_out[b][ib][:]],
                    replica_groups=[list(range(NCORES))],
                )

        # ================= schedule =================
        # Phase A0: QKV for batch 0 (blocks 0-3), serial (DMA-paced warmup).
        for blk in range(4):
            for name in ("q", "k", "v"):
                qkv_chain(blk, name)

        # b0 attention streams b1's QKV (k-chains first: b1's S steps
        # need them immediately at the b0->b1 boundary).
        s0 = Stream()
        s0.add(lambda: issue_xb(4), lambda: issue_xb(5))
        for blk in (4, 5):
            s0.add(*qkv_chain_units(blk, "k"))
        s0.add(lambda: issue_xb(6), lambda: issue_xb(7))
        for blk in (6, 7):
            s0.add(*qkv_chain_units(blk, "k"))
        for blk in range(4, 8):
            s0.add(*qkv_chain_units(blk, "q"))
        for blk in range(4, 8):
            s0.add(*qkv_chain_units(blk, "v"))
        attn_chunk(0, 0, s0)
        attn_chunk(0, 1, s0)
        s0.drain()

        # b1 attention streams the batch-0 projection.
        s1 = Stream()
        s1.add(lambda: issue_ab(0, 0))
        s1.add(*proj_chain_units(0, 0, 0))
        s1.add(*proj_chain_units(0, 0, 1))
        s1.add(lambda: issue_ab(0, 1))
        s1.add(*proj_chain_units(0, 1, 0))
        s1.add(*proj_chain_units(0, 1, 1))
        attn_chunk(1, 0, s1, pull_every=2)
        s1.add(lambda: issue_ab(1, 0))
        attn_chunk(1, 1, s1, pull_every=2, fast_stage=True)
        s1.drain()

        # tail: batch-1 projection. The last chunk was gathered per-head,
        # so its projection contracts 4 kc chunks from each gathered half
        # (w_p2a/w_p2b rows are permuted to match).
        proj_chain(1, 0, 0)
        proj_chain(1, 0, 1)
        ab3 = []
        for h in range(2):
            ag3r = ag_out3[h].rearrange("(kc p) t -> p kc t", p=128)
            a3 = apool.tile([128, KC // 2, HALF], proj_dt, tag=f"ab3_{h}")
            nc.gpsimd.dma_start(out=a3, in_=ag3r)
            ab3.append(a3)
        for i2 in range(2):
            i0 = i2 * 512
            ps = ps_mm.tile([128, 512], F32, tag="mmA")
            for kc in range(KC // 2):
                nc.tensor.matmul(
                    out=ps, lhsT=w_p2a[:, kc, :], rhs=ab3[0][:, kc, i0 : i0 + 512],
                    start=(kc == 0), stop=False,
                )
            for kc in range(KC // 2):
                nc.tensor.matmul(
                    out=ps, lhsT=w_p2b[:, kc, :], rhs=ab3[1][:, kc, i0 : i0 + 512],
                    start=False, stop=(kc == KC // 2 - 1),
                )
            ot = oupool.tile([128, 512], F32, tag="ot")
            nc.vector.tensor_scalar_add(out=ot, in0=ps, scalar1=b_p)
            to = N + HALF + i0
            nc.sync.dma_start(out=out[:, to : to + 512], in_=ot)

    nc.compile()
    return nc


def np_dt(dt):
    return mybir.dt.np(F32 if dt == F32R else dt)


def prep_in_maps(x, Wqkv, bqkv, Wproj, bproj, qkv_dt=F32R, attn_dt=F32R,
                 proj_dt=BF16):
    x = np.asarray(x, dtype=np.float32)
    Wqkv = np.asarray(Wqkv, dtype=np.float32)
    bqkv = np.asarray(bqkv, dtype=np.float32)
    Wproj = np.asarray(Wproj, dtype=np.float32)
    bproj = np.asarray(bproj, dtype=np.float32)

    # x^T block-major: [128, blk, kc, 512]; row kc*128+p of x^T.
    xT = np.ascontiguousarray(x.reshape(TOK, D).T)     # [D, TOK]
    xtn = np.ascontiguousarray(
        xT.reshape(KC, 128, NBLK, 512).transpose(1, 2, 0, 3)
    ).astype(np_dt(qkv_dt))
    identity = np.eye(128, dtype=np_dt(attn_dt))
    ones_col = np.ones((128, 1), dtype=np_dt(attn_dt))

    def perm_w(w):  # [D', 128] -> [128, D'//128, 128] with row kc*128+p
        return np.ascontiguousarray(
            w.reshape(-1, 128, w.shape[1]).transpose(1, 0, 2)
        )

    # AllGather output rows are rank-major: row c*128 + hl*64 + d holds
    # feature (2c+hl)*64 + d; permute Wproj's contraction rows to match.
    wp_row_perm = np.empty(D, dtype=np.int64)
    for cc in range(NCORES):
        for hlhl in range(2):
            rows = np.arange(64)
            wp_row_perm[cc * 128 + hlhl * 64 + rows] = (2 * cc + hlhl) * 64 + rows

    # Split-gather permutations: gathered half h row r = c*64 + d holds
    # feature (2c+h)*64 + d; contraction chunk j covers cores 2j, 2j+1.
    wp2_perm = [np.empty(D // 2, dtype=np.int64) for _ in range(2)]
    for hh in range(2):
        for cc in range(NCORES):
            rows = np.arange(64)
            wp2_perm[hh][cc * 64 + rows] = (2 * cc + hh) * 64 + rows

    # qkv column index for (head h, depth d, which): h*192 + d*3 + which
    d_idx = np.arange(DEPTH)
    in_maps = []
    for c in range(NCORES):
        h0, h1 = 2 * c, 2 * c + 1
        qcols = np.concatenate([h0 * 192 + 3 * d_idx, h1 * 192 + 3 * d_idx])
        kcols = qcols + 1
        vcols = qcols + 2
        in_maps.append(
            {
                "xt": xtn,
                "wq": perm_w(Wqkv[:, qcols]).astype(np_dt(qkv_dt)),
                "wk": perm_w(Wqkv[:, kcols]).astype(np_dt(qkv_dt)),
                "wv": perm_w(Wqkv[:, vcols]).astype(np_dt(qkv_dt)),
                "wp": perm_w(
                    Wproj[wp_row_perm, 128 * c : 128 * (c + 1)]
                ).astype(np_dt(proj_dt)),
                "wp2a": perm_w(
                    Wproj[wp2_perm[0], 128 * c : 128 * (c + 1)]
                ).astype(np_dt(proj_dt)),
                "wp2b": perm_w(
                    Wproj[wp2_perm[1], 128 * c : 128 * (c + 1)]
                ).astype(np_dt(proj_dt)),
                "bq": np.ascontiguousarray(bqkv[qcols]).reshape(128, 1),
                "bk": np.ascontiguousarray(bqkv[kcols]).reshape(128, 1),
                "bv": np.ascontiguousarray(bqkv[vcols]).reshape(128, 1),
                "bp": np.ascontiguousarray(
                    bproj[128 * c : 128 * (c + 1)]
                ).reshape(128, 1),
                "ident": identity,
                "ones": ones_col,
            }
        )
    return in_maps


def assemble(results):
    outT = np.concatenate([r["o"] for r in results], axis=0)  # [D, TOK]
    return np.ascontiguousarray(outT.T).reshape(B, N, D).astype(np.float32)


CONFIG = {"qkv_dt": BF16, "attn_dt": F32R, "proj_dt": BF16}

_NC_CACHE = {}


def get_nc():
    if "nc" not in _NC_CACHE:
        _NC_CACHE["nc"] = build_nc(**CONFIG)
    return _NC_CACHE["nc"]


def kernel(x, Wqkv, bqkv, Wproj, bproj):
    nc = get_nc()
    in_maps = prep_in_maps(x, Wqkv, bqkv, Wproj, bproj, **CONFIG)
    res = run_bass_kernel_spmd(nc, in_maps, list(range(NCORES)))
    return assemble(res.results)


# revision 29
# speedup vs baseline: 1.8540x; 1.0337x over previous
"""Multi-head attention TRN2 kernel (B=2, N=2048, D=1024, H=16).

Sharding: tensor-parallel over heads. Each of the 8 cores owns 2 heads
(both batch elements) end-to-end through QKV projection and attention,
then the per-head attention outputs are AllGathered (bf16, per batch
half) and each core computes a 128-column slice of the output
projection.

Dtypes: x and the QKV weights are bf16 (halves the input streaming);
the attention matmuls run in float32r (same 1 cyc/row PE rate as bf16
for >=256-wide outputs, and the bf16 S/PV/transpose path miscompiles on
HW); the projection path (AllGather payload, Wproj, projection matmul)
is bf16. Net ~4e-3 scaled error, 5x under the 2e-2 gate.

Softmax runs without max-subtraction (scores are O(5); exp is safe in
fp32): S^T is computed key-major via matmul(lhsT=kT, rhs=qT), exp'd on
ScalarE, and the denominator comes from a ones-column appended to V in
the P^T@V matmul. The S->exp->PV software pipeline is two steps deep.
Normalization happens off the PE path: one [65,1024] DVE copy releases
PSUM, then approx-reciprocal + a DRAM broadcast round-trip + multiply
(all on DVE/GpSimd/sync queues) feed the AllGather staging.

The attention phase is ScalarE-paced (exp of a [128,1024] chunk takes
~1.1us vs ~0.93us of PE work per step), and on TRN2 every PE stall
resets the tensor engine's DVFS ramp, halving its clock. So all other
matmul work is split into ~2-matmul micro-units that attention steps
pull BETWEEN their S and PV matmuls (the exact spot where the PE would
otherwise stall on exp): batch-1 QKV chains stream through batch-0's
attention, the batch-0 projection streams through batch-1's attention,
and input DMA + AllGathers overlap attention compute throughout.

The last attention chunk uses a low-latency staging variant: the
reciprocal row is replicated across partitions with a rank-1 matmul
(no DRAM round trip, ~8us from last PV to the AllGather trigger), and
its AllGather is split per head group so the first head's gather flies
while the second head is still computing. The final projection
contracts 4 kc chunks from each per-head gathered half (w_p2a/w_p2b
carry the matching row permutations).

All host-side tensors are laid out so every DMA descriptor is >=2KB
contiguous per partition (strided layouts ran the HBM queues at
~75GB/s; these run near peak).

Self-contained: hardcodes shapes from the problem spec.
"""

import sys

for _p in ("/opt/trn_rl_repo", "/root/.axon_site/_ro/trn_rl_repo"):
    if _p not in sys.path:
        sys.path.append(_p)

import numpy as np
from contextlib import ExitStack

import concourse.bass as bass
import concourse.tile as tile
from concourse import mybir, bacc
from concourse.bass_utils import run_bass_kernel_spmd

F32 = mybir.dt.float32
F32R = mybir.dt.float32r
BF16 = mybir.dt.bfloat16
EXP = mybir.ActivationFunctionType.Exp

B = 2
N = 2048
D = 1024
H = 16
DEPTH = 64
TOK = B * N            # 4096 tokens total (both batches)
KC = D // 128          # 8 contraction chunks of 128
NBLK = TOK // 512      # 8 token blocks for streaming projections
SCALE = 1.0 / np.sqrt(DEPTH)
NCORES = 8
IBLK = 1024            # query-block width in attention
NSUB = IBLK // 512     # matmuls per psum tile (N<=512 for 4-byte dtypes)
HALF = N // 2
NJC = N // 128         # key chunks per batch


def build_nc(qkv_dt=F32R, attn_dt=F32R, proj_dt=BF16):
    nc = bacc.Bacc(None)

    def dram_dt(dt):
        return F32 if dt == F32R else dt

    def cast(ap, dt):
        return ap.bitcast(F32R) if dt == F32R else ap

    # x^T stored block-major: [128, blk, kc, 512] so each 512-token block
    # is 16KB contiguous per partition.
    xt = nc.dram_tensor("xt", [128, NBLK, KC, 512], dram_dt(qkv_dt),
                        kind="ExternalInput")
    # weights pre-permuted to [128, kc, 128] (4KB contiguous/partition)
    wq = nc.dram_tensor("wq", [128, KC, 128], dram_dt(qkv_dt), kind="ExternalInput")
    wk = nc.dram_tensor("wk", [128, KC, 128], dram_dt(qkv_dt), kind="ExternalInput")
    wv = nc.dram_tensor("wv", [128, KC, 128], dram_dt(qkv_dt), kind="ExternalInput")
    wp = nc.dram_tensor("wp", [128, KC, 128], dram_dt(proj_dt), kind="ExternalInput")
    bq = nc.dram_tensor("bq", [128, 1], F32, kind="ExternalInput")
    bk = nc.dram_tensor("bk", [128, 1], F32, kind="ExternalInput")
    bv = nc.dram_tensor("bv", [128, 1], F32, kind="ExternalInput")
    bp = nc.dram_tensor("bp", [128, 1], F32, kind="ExternalInput")
    ident = nc.dram_tensor(
        "ident", [128, 128], dram_dt(attn_dt), kind="ExternalInput"
    )
    ones = nc.dram_tensor("ones", [128, 1], dram_dt(attn_dt), kind="ExternalInput")
    out = nc.dram_tensor("o", [128, TOK], F32, kind="ExternalOutput")

    # Collective staging (split per (batch, half) so early AllGathers
    # overlap later attention).
    ag_in = [
        [nc.dram_tensor(f"ag_in{b}_{h}", [128, HALF], proj_dt) for h in range(2)]
        for b in range(B)
    ]
    ag_out = [
        [
            nc.dram_tensor(f"ag_out{b}_{h}", [D, HALF], proj_dt,
                           addr_space="Shared")
            for h in range(2)
        ]
        for b in range(B)
    ]
    # split staging for the LAST chunk: one gather per head group, so the
    # first head's AllGather overlaps the second head's attention.
    ag_in3 = [nc.dram_tensor(f"ag_in3_{h}", [64, HALF], proj_dt)
              for h in range(2)]
    ag_out3 = [nc.dram_tensor(f"ag_out3_{h}", [D // 2, HALF], proj_dt,
                              addr_space="Shared") for h in range(2)]
    wp2a = nc.dram_tensor("wp2a", [128, KC // 2, 128], dram_dt(proj_dt),
                          kind="ExternalInput")
    wp2b = nc.dram_tensor("wp2b", [128, KC // 2, 128], dram_dt(proj_dt),
                          kind="ExternalInput")

    with tile.TileContext(nc) as tc, ExitStack() as ctx:
        wpool = ctx.enter_context(tc.tile_pool(name="w", bufs=1))
        qkpool = ctx.enter_context(tc.tile_pool(name="qk", bufs=1))
        vpool = ctx.enter_context(tc.tile_pool(name="v2", bufs=1))
        xpool = ctx.enter_context(tc.tile_pool(name="x", bufs=4))
        vtpool = ctx.enter_context(tc.tile_pool(name="vt", bufs=2))
        ptpool = ctx.enter_context(tc.tile_pool(name="pt", bufs=3))
        unpool = ctx.enter_context(tc.tile_pool(name="un", bufs=2))
        rpool = ctx.enter_context(tc.tile_pool(name="r", bufs=1))
        rdpool = ctx.enter_context(tc.tile_pool(name="rd", bufs=2, space="DRAM"))
        apool = ctx.enter_context(tc.tile_pool(name="ap", bufs=2))
        oupool = ctx.enter_context(tc.tile_pool(name="ou", bufs=2))
        # PSUM budget (8 banks of 2KB/partition):
        #   ps_one (bufs=1): po [65,1024]                -> 2 banks
        #   ps_mm (bufs=2): mmA [128,512] qkv/transpose/proj -> 2 banks
        #   ps_two (bufs=2): ss [128,1024]               -> 4 banks
        ps_one = ctx.enter_context(tc.tile_pool(name="ps1", bufs=1, space="PSUM"))
        ps_mm = ctx.enter_context(tc.tile_pool(name="psm", bufs=2, space="PSUM"))
        ps_two = ctx.enter_context(tc.tile_pool(name="ps2", bufs=2, space="PSUM"))

        # ---- weights / constants ----
        w_q = wpool.tile([128, KC, 128], qkv_dt, tag="w_q")
        w_k = wpool.tile([128, KC, 128], qkv_dt, tag="w_k")
        w_v = wpool.tile([128, KC, 128], qkv_dt, tag="w_v")
        w_p = wpool.tile([128, KC, 128], proj_dt, tag="w_p")
        w_p2a = wpool.tile([128, KC // 2, 128], proj_dt, tag="w_p2a")
        w_p2b = wpool.tile([128, KC // 2, 128], proj_dt, tag="w_p2b")
        b_q = wpool.tile([128, 1], F32, tag="b_q")
        b_k = wpool.tile([128, 1], F32, tag="b_k")
        b_v = wpool.tile([128, 1], F32, tag="b_v")
        b_p = wpool.tile([128, 1], F32, tag="b_p")
        id_t = wpool.tile([128, 128], attn_dt, tag="id_t")

        # qT/kT: [feature 128 (= 2 heads x 64), token 4096]; head hl in rows
        # hl*64:(hl+1)*64 so both S^T operands share a partition base.
        qT = qkpool.tile([128, TOK], attn_dt, tag="qT")
        kT = qkpool.tile([128, TOK], attn_dt, tag="kT")
        # V2: [token part, 32 token-chunks, 130]: v_h0 | ones | v_h1 | ones
        V2 = vpool.tile([128, TOK // 128, 130], attn_dt, tag="V2")

        # ---- startup DMA order: xb0 (gpsimd) + w_q (sync) in parallel
        # so the first QKV chain can start at ~20us.
        xbs = {}

        def issue_xb(blk):
            xb = xpool.tile([128, KC, 512], qkv_dt, tag="xb")
            eng = nc.sync if blk % 2 == 0 else nc.gpsimd
            eng.dma_start(out=xb, in_=cast(xt[:, blk], qkv_dt))
            xbs[blk] = xb

        nc.gpsimd.dma_start(out=b_q, in_=bq[:])
        xb0t = xpool.tile([128, KC, 512], qkv_dt, tag="xb")
        nc.gpsimd.dma_start(out=xb0t[:, 0:4], in_=cast(xt[:, 0, 0:4], qkv_dt))
        nc.sync.dma_start(out=w_q, in_=cast(wq[:], qkv_dt))
        nc.sync.dma_start(out=xb0t[:, 4:8], in_=cast(xt[:, 0, 4:8], qkv_dt))
        xbs[0] = xb0t
        nc.sync.dma_start(out=w_k, in_=cast(wk[:], qkv_dt))
        nc.sync.dma_start(out=w_v, in_=cast(wv[:], qkv_dt))
        for t, src in ((b_k, bk), (b_v, bv)):
            nc.gpsimd.dma_start(out=t, in_=src[:])
        nc.gpsimd.dma_start(out=id_t, in_=cast(ident[:], attn_dt))
        issue_xb(1)
        issue_xb(2)
        issue_xb(3)
        nc.gpsimd.dma_start(out=w_p, in_=cast(wp[:], proj_dt))
        nc.gpsimd.dma_start(out=w_p2a, in_=cast(wp2a[:], proj_dt))
        nc.gpsimd.dma_start(out=w_p2b, in_=cast(wp2b[:], proj_dt))
        nc.gpsimd.dma_start(out=b_p, in_=bp[:])
        # ones column of V2 via one tiny DMA + free-axis broadcast copies
        # (a broadcast DMA would generate 8192 4-byte descriptors).
        ones_r = wpool.tile([1, 128], attn_dt, tag="ones_r")
        nc.scalar.dma_start(
            out=ones_r, in_=cast(ones[:].rearrange("p o -> o p"), attn_dt)
        )
        nc.scalar.dma_start(
            out=V2[:, :, 64:65],
            in_=cast(ones[:].to_broadcast((128, TOK // 128, 1)), attn_dt),
        )
        nc.scalar.dma_start(
            out=V2[:, :, 129:130],
            in_=cast(ones[:].to_broadcast((128, TOK // 128, 1)), attn_dt),
        )

        # ---- work-unit emitters ----
        # Filler work (QKV chains, projection chains) is split into
        # micro-units of ~2 matmuls. Attention groups pull one unit
        # BETWEEN the S and PV matmuls of a step, so the PE has
        # independent work exactly where it would otherwise stall
        # waiting for ScalarE's exp — keeping the tensor engine
        # continuously busy (and at full DVFS clock).
        class Stream:
            def __init__(self):
                self.units = []

            def add(self, *units):
                self.units.extend(units)

            def pull(self):
                if self.units:
                    self.units.pop(0)()

            def drain(self):
                while self.units:
                    self.units.pop(0)()

        def qkv_chain_units(blk, name):
            """Micro-units for one q/k/v projection chain of a block."""
            w_t, b_t = {"q": (w_q, b_q), "k": (w_k, b_k), "v": (w_v, b_v)}[name]
            t0 = blk * 512
            state = {}

            def mk_mm(k0, k1):
                def f():
                    if "ps" not in state:
                        pst = ps_mm.tile([128, 512], F32, tag="mmA")
                        state["ps"] = pst
                    xb = xbs[blk]
                    for kc in range(k0, k1):
                        nc.tensor.matmul(
                            out=state["ps"], lhsT=w_t[:, kc, :],
                            rhs=xb[:, kc, :],
                            start=(kc == 0), stop=(kc == KC - 1),
                        )
                return f

            def fin():
                ps = state["ps"]
                if name == "q":
                    nc.vector.tensor_scalar_add(
                        out=qT[:, t0 : t0 + 512], in0=ps, scalar1=b_t
                    )
                elif name == "k":
                    nc.vector.tensor_scalar_add(
                        out=kT[:, t0 : t0 + 512], in0=ps, scalar1=b_t
                    )
                else:
                    vtmp = vtpool.tile([128, 512], attn_dt, tag="vtmp")
                    nc.vector.tensor_scalar_add(out=vtmp, in0=ps, scalar1=b_t)
                    state["vtmp"] = vtmp

            def mk_tr(sx):
                def f():
                    vtmp = state["vtmp"]
                    ch = blk * 4 + sx
                    ps_t = ps_mm.tile([128, 128], attn_dt, tag="mmA")
                    nc.tensor.transpose(
                        out=ps_t, in_=vtmp[:, sx * 128 : (sx + 1) * 128],
                        identity=id_t,
                    )
                    nc.vector.tensor_copy(out=V2[:, ch, 0:64], in_=ps_t[:, 0:64])
                    nc.vector.tensor_copy(
                        out=V2[:, ch, 65:129], in_=ps_t[:, 64:128]
                    )
                return f

            units = [mk_mm(0, 4), mk_mm(4, 8), fin]
            if name == "v":
                units += [mk_tr(sx) for sx in range(4)]
            return units

        def qkv_chain(blk, name):
            for u in qkv_chain_units(blk, name):
                u()

        abs_ = {}

        def issue_ab(b, hf):
            ag_r = ag_out[b][hf].rearrange("(kc p) t -> p kc t", p=128)
            ab = apool.tile([128, KC, HALF], proj_dt, tag="ab")
            nc.gpsimd.dma_start(out=ab, in_=ag_r)
            abs_[(b, hf)] = ab

        def proj_chain_units(b, hf, i2):
            i0 = i2 * 512
            state = {}

            def mk_mm(k0, k1):
                def f():
                    if "ps" not in state:
                        pst = ps_mm.tile([128, 512], F32, tag="mmA")
                        state["ps"] = pst
                    ab = abs_[(b, hf)]
                    for kc in range(k0, k1):
                        nc.tensor.matmul(
                            out=state["ps"], lhsT=w_p[:, kc, :],
                            rhs=ab[:, kc, i0 : i0 + 512],
                            start=(kc == 0), stop=(kc == KC - 1),
                        )
                return f

            def fin():
                ot = oupool.tile([128, 512], F32, tag="ot")
                nc.vector.tensor_scalar_add(out=ot, in0=state["ps"], scalar1=b_p)
                to = b * N + hf * HALF + i0
                nc.sync.dma_start(out=out[:, to : to + 512], in_=ot)

            return [mk_mm(0, 4), mk_mm(4, 8), fin]

        def proj_chain(b, hf, i2):
            for u in proj_chain_units(b, hf, i2):
                u()

        def attn_group(b, ib, hl, stream, pull_every=1, fast_stage=False):
            """One (batch, query-block, head) attention group. Pulls one
            stream unit between the S and PV matmuls of each step."""
            hs = hl * 64
            voff = hl * 65
            i0 = b * N + ib * IBLK
            ps_o = ps_one.tile([65, IBLK], F32, tag="po")

            def s_step(jc):
                j0 = b * N + jc * 128
                ps_s = ps_two.tile([128, IBLK], F32, tag="ss")
                for su in range(NSUB):
                    nc.tensor.matmul(
                        out=ps_s[:, su * 512 : (su + 1) * 512],
                        lhsT=kT[hs : hs + 64, j0 : j0 + 128],
                        rhs=qT[hs : hs + 64, i0 + su * 512 : i0 + (su + 1) * 512],
                        start=True, stop=True,
                    )
                pt = ptpool.tile([128, IBLK], attn_dt, tag="pt")
                nc.scalar.activation(out=pt, in_=ps_s, func=EXP, scale=float(SCALE))
                return pt

            pt_q = [s_step(0), s_step(1)]
            for jc in range(NJC):
                pt_cur = pt_q.pop(0)
                if jc + 2 < NJC:
                    pt_q.append(s_step(jc + 2))
                if jc % pull_every == 0:
                    stream.pull()
                for su in range(NSUB):
                    nc.tensor.matmul(
                        out=ps_o[:, su * 512 : (su + 1) * 512],
                        lhsT=V2[:, ((b * N + jc * 128) // 128), voff : voff + 65],
                        rhs=pt_cur[:, su * 512 : (su + 1) * 512],
                        start=(jc == 0), stop=(jc == NJC - 1),
                    )
            # single copy (rows + denominator) releases PSUM.
            und = unpool.tile([65, IBLK], F32, tag="und")
            nc.vector.tensor_copy(out=und, in_=ps_o)
            unr = unpool.tile([64, IBLK], proj_dt, tag="unr")
            if fast_stage:
                # recip of the whole tile (same DVE cost as one row), then
                # replicate the denominator row across 64 partitions with
                # a rank-1 matmul: ~8us from last PV to the AG trigger.
                rc65 = rpool.tile([65, IBLK], F32, tag="rc65")
                nc.vector.reciprocal_approx_fast(out=rc65, in_=und)
                rcr = rpool.tile([1, IBLK], attn_dt, tag="rcr")
                nc.vector.tensor_copy(out=rcr, in_=rc65[64:65, :])
                for su in range(NSUB):
                    ps_bc = ps_mm.tile([64, 512], F32, tag="mmA")
                    nc.tensor.matmul(
                        out=ps_bc,
                        lhsT=ones_r[:, 0:64],
                        rhs=rcr[:, su * 512 : (su + 1) * 512],
                        start=True, stop=True,
                    )
                    nc.vector.tensor_mul(
                        out=unr[:, su * 512 : (su + 1) * 512],
                        in0=und[0:64, su * 512 : (su + 1) * 512],
                        in1=ps_bc,
                    )
                nc.sync.dma_start(out=ag_in3[hl][:], in_=unr)
                nc.gpsimd.collective_compute(
                    "AllGather", mybir.AluOpType.bypass,
                    ins=[ag_in3[hl][:]], outs=[ag_out3[hl][:]],
                    replica_groups=[list(range(NCORES))],
                )
            else:
                rd = rdpool.tile([1, IBLK], F32, tag="rd")
                nc.gpsimd.dma_start(out=rd, in_=und[64:65, :])
                rr = rpool.tile([64, IBLK], F32, tag="rr")
                nc.gpsimd.dma_start(out=rr, in_=rd.to_broadcast((64, IBLK)))
                rcp = rpool.tile([64, IBLK], F32, tag="rcp")
                nc.vector.reciprocal_approx_fast(out=rcp, in_=rr)
                nc.gpsimd.tensor_mul(out=unr, in0=und[0:64, :], in1=rcp)
                nc.gpsimd.dma_start(out=ag_in[b][ib][hs : hs + 64, :], in_=unr)

        def attn_chunk(b, ib, stream, pull_every=1, fast_stage=False):
            attn_group(b, ib, 0, stream, pull_every, fast_stage)
            attn_group(b, ib, 1, stream, pull_every, fast_stage)
            if not fast_stage:
                nc.gpsimd.collective_compute(
                    "AllGather", mybir.AluOpType.bypass,
                    ins=[ag_in[b][ib][:]], outs=[ag_out[b][ib][:]],
                    replica_groups=[list(range(NCORES))],
                )

        # ================= schedule =================
        # Phase A0: QKV for batch 0 (blocks 0-3), serial (DMA-paced warmup).
        for blk in range(4):
            for name in ("q", "k", "v"):
                qkv_chain(blk, name)

        # b0 attention streams b1's QKV (k-chains first: b1's S steps
        # need them immediately at the b0->b1 boundary).
        s0 = Stream()
        s0.add(lambda: issue_xb(4), lambda: issue_xb(5))
        for blk in (4, 5):
            s0.add(*qkv_chain_units(blk, "k"))
        s0.add(lambda: issue_xb(6), lambda: issue_xb(7))
        for blk in (6, 7):
            s0.add(*qkv_chain_units(blk, "k"))
        for blk in range(4, 8):
            s0.add(*qkv_chain_units(blk, "q"))
        for blk in range(4, 8):
            s0.add(*qkv_chain_units(blk, "v"))
        attn_chunk(0, 0, s0)
        attn_chunk(0, 1, s0)
        s0.drain()

        # b1 attention streams the batch-0 projection.
        s1 = Stream()
        s1.add(lambda: issue_ab(0, 0))
        s1.add(*proj_chain_units(0, 0, 0))
        s1.add(*proj_chain_units(0, 0, 1))
        s1.add(lambda: issue_ab(0, 1))
        s1.add(*proj_chain_units(0, 1, 0))
        s1.add(*proj_chain_units(0, 1, 1))
        attn_chunk(1, 0, s1, pull_every=1)
        s1.add(lambda: issue_ab(1, 0))
        attn_chunk(1, 1, s1, pull_every=1, fast_stage=True)
        s1.drain()

        # tail: batch-1 projection. The last chunk was gathered per-head,
        # so its projection contracts 4 kc chunks from each gathered half
        # (w_p2a/w_p2b rows are permuted to match).
        proj_chain(1, 0, 0)
        proj_chain(1, 0, 1)
        ab3 = []
        for h in range(2):
            ag3r = ag_out3[h].rearrange("(kc p) t -> p kc t", p=128)
            a3 = apool.tile([128, KC // 2, HALF], proj_dt, tag=f"ab3_{h}")
            nc.gpsimd.dma_start(out=a3, in_=ag3r)
            ab3.append(a3)
        for i2 in range(2):
            i0 = i2 * 512
            ps = ps_mm.tile([128, 512], F32, tag="mmA")
            for kc in range(KC // 2):
                nc.tensor.matmul(
                    out=ps, lhsT=w_p2a[:, kc, :], rhs=ab3[0][:, kc, i0 : i0 + 512],
                    start=(kc == 0), stop=False,
                )
            for kc in range(KC // 2):
                nc.tensor.matmul(
                    out=ps, lhsT=w_p2b[:, kc, :], rhs=ab3[1][:, kc, i0 : i0 + 512],
                    start=False, stop=(kc == KC // 2 - 1),
                )
            ot = oupool.tile([128, 512], F32, tag="ot")
            nc.vector.tensor_scalar_add(out=ot, in0=ps, scalar1=b_p)
            to = N + HALF + i0
            nc.sync.dma_start(out=out[:, to : to + 512], in_=ot)

    nc.compile()
    return nc


def np_dt(dt):
    return mybir.dt.np(F32 if dt == F32R else dt)


def prep_in_maps(x, Wqkv, bqkv, Wproj, bproj, qkv_dt=F32R, attn_dt=F32R,
                 proj_dt=BF16):
    x = np.asarray(x, dtype=np.float32)
    Wqkv = np.asarray(Wqkv, dtype=np.float32)
    bqkv = np.asarray(bqkv, dtype=np.float32)
    Wproj = np.asarray(Wproj, dtype=np.float32)
    bproj = np.asarray(bproj, dtype=np.float32)

    # x^T block-major: [128, blk, kc, 512]; row kc*128+p of x^T.
    xT = np.ascontiguousarray(x.reshape(TOK, D).T)     # [D, TOK]
    xtn = np.ascontiguousarray(
        xT.reshape(KC, 128, NBLK, 512).transpose(1, 2, 0, 3)
    ).astype(np_dt(qkv_dt))
    identity = np.eye(128, dtype=np_dt(attn_dt))
    ones_col = np.ones((128, 1), dtype=np_dt(attn_dt))

    def perm_w(w):  # [D', 128] -> [128, D'//128, 128] with row kc*128+p
        return np.ascontiguousarray(
            w.reshape(-1, 128, w.shape[1]).transpose(1, 0, 2)
        )

    # AllGather output rows are rank-major: row c*128 + hl*64 + d holds
    # feature (2c+hl)*64 + d; permute Wproj's contraction rows to match.
    wp_row_perm = np.empty(D, dtype=np.int64)
    for cc in range(NCORES):
        for hlhl in range(2):
            rows = np.arange(64)
            wp_row_perm[cc * 128 + hlhl * 64 + rows] = (2 * cc + hlhl) * 64 + rows

    # Split-gather permutations: gathered half h row r = c*64 + d holds
    # feature (2c+h)*64 + d; contraction chunk j covers cores 2j, 2j+1.
    wp2_perm = [np.empty(D // 2, dtype=np.int64) for _ in range(2)]
    for hh in range(2):
        for cc in range(NCORES):
            rows = np.arange(64)
            wp2_perm[hh][cc * 64 + rows] = (2 * cc + hh) * 64 + rows

    # qkv column index for (head h, depth d, which): h*192 + d*3 + which
    d_idx = np.arange(DEPTH)
    in_maps = []
    for c in range(NCORES):
        h0, h1 = 2 * c, 2 * c + 1
        qcols = np.concatenate([h0 * 192 + 3 * d_idx, h1 * 192 + 3 * d_idx])
        kcols = qcols + 1
        vcols = qcols + 2
        in_maps.append(
            {
                "xt": xtn,
                "wq": perm_w(Wqkv[:, qcols]).astype(np_dt(qkv_dt)),
                "wk": perm_w(Wqkv[:, kcols]).astype(np_dt(qkv_dt)),
                "wv": perm_w(Wqkv[:, vcols]).astype(np_dt(qkv_dt)),
                "wp": perm_w(
                    Wproj[wp_row_perm, 128 * c : 128 * (c + 1)]
                ).astype(np_dt(proj_dt)),
                "wp2a": perm_w(
                    Wproj[wp2_perm[0], 128 * c : 128 * (c + 1)]
                ).astype(np_dt(proj_dt)),
                "wp2b": perm_w(
                    Wproj[wp2_perm[1], 128 * c : 128 * (c + 1)]
                ).astype(np_dt(proj_dt)),
                "bq": np.ascontiguousarray(bqkv[qcols]).reshape(128, 1),
                "bk": np.ascontiguousarray(bqkv[kcols]).reshape(128, 1),
                "bv": np.ascontiguousarray(bqkv[vcols]).reshape(128, 1),
                "bp": np.ascontiguousarray(
                    bproj[128 * c : 128 * (c + 1)]
                ).reshape(128, 1),
                "ident": identity,
                "ones": ones_col,
            }
        )
    return in_maps


def assemble(results):
    outT = np.concatenate([r["o"] for r in results], axis=0)  # [D, TOK]
    return np.ascontiguousarray(outT.T).reshape(B, N, D).astype(np.float32)


CONFIG = {"qkv_dt": BF16, "attn_dt": F32R, "proj_dt": BF16}

_NC_CACHE = {}


def get_nc():
    if "nc" not in _NC_CACHE:
        _NC_CACHE["nc"] = build_nc(**CONFIG)
    return _NC_CACHE["nc"]


def kernel(x, Wqkv, bqkv, Wproj, bproj):
    nc = get_nc()
    in_maps = prep_in_maps(x, Wqkv, bqkv, Wproj, bproj, **CONFIG)
    res = run_bass_kernel_spmd(nc, in_maps, list(range(NCORES)))
    return assemble(res.results)


# revision 30
# speedup vs baseline: 1.9686x; 1.0618x over previous
"""Multi-head attention TRN2 kernel (B=2, N=2048, D=1024, H=16).

Sharding: tensor-parallel over heads. Each of the 8 cores owns 2 heads
(both batch elements) end-to-end through QKV projection and attention,
then the per-head attention outputs are AllGathered (bf16, per batch
half) and each core computes a 128-column slice of the output
projection.

Dtypes: x and the QKV weights are bf16 (halves the input streaming);
the attention matmuls run in float32r (same 1 cyc/row PE rate as bf16
for >=256-wide outputs, and the bf16 S/PV/transpose path miscompiles on
HW); the projection path (AllGather payload, Wproj, projection matmul)
is bf16. Net ~4e-3 scaled error, 5x under the 2e-2 gate.

Softmax runs without max-subtraction (scores are O(5); exp is safe in
fp32): S^T is computed key-major via matmul(lhsT=kT, rhs=qT), exp'd on
ScalarE, and the denominator comes from a ones-column appended to V in
the P^T@V matmul. The S->exp->PV software pipeline is two steps deep.
Normalization happens off the PE path: one [65,1024] DVE copy releases
PSUM, then approx-reciprocal + a DRAM broadcast round-trip + multiply
(all on DVE/GpSimd/sync queues) feed the AllGather staging.

The attention phase is ScalarE-paced (exp of a [128,1024] chunk takes
~1.1us vs ~0.93us of PE work per step), and on TRN2 every PE stall
resets the tensor engine's DVFS ramp, halving its clock. So all other
matmul work is split into ~2-matmul micro-units that attention steps
pull BETWEEN their S and PV matmuls (the exact spot where the PE would
otherwise stall on exp): batch-1 QKV chains stream through batch-0's
attention, the batch-0 projection streams through batch-1's attention,
and input DMA + AllGathers overlap attention compute throughout.

The last attention chunk uses a low-latency staging variant: the
reciprocal row is replicated across partitions with a rank-1 matmul
(no DRAM round trip, ~8us from last PV to the AllGather trigger), and
its AllGather is split per head group so the first head's gather flies
while the second head is still computing. The final projection
contracts 4 kc chunks from each per-head gathered half (w_p2a/w_p2b
carry the matching row permutations).

All host-side tensors are laid out so every DMA descriptor is >=2KB
contiguous per partition (strided layouts ran the HBM queues at
~75GB/s; these run near peak).

Self-contained: hardcodes shapes from the problem spec.
"""

import sys

for _p in ("/opt/trn_rl_repo", "/root/.axon_site/_ro/trn_rl_repo"):
    if _p not in sys.path:
        sys.path.append(_p)

import numpy as np
from contextlib import ExitStack

import concourse.bass as bass
import concourse.tile as tile
from concourse import mybir, bacc
from concourse.bass_utils import run_bass_kernel_spmd

F32 = mybir.dt.float32
F32R = mybir.dt.float32r
BF16 = mybir.dt.bfloat16
EXP = mybir.ActivationFunctionType.Exp

B = 2
N = 2048
D = 1024
H = 16
DEPTH = 64
TOK = B * N            # 4096 tokens total (both batches)
KC = D // 128          # 8 contraction chunks of 128
NBLK = TOK // 512      # 8 token blocks for streaming projections
SCALE = 1.0 / np.sqrt(DEPTH)
NCORES = 8
IBLK = 1024            # query-block width in attention
NSUB = IBLK // 512     # matmuls per psum tile (N<=512 for 4-byte dtypes)
HALF = N // 2
NJC = N // 128         # key chunks per batch


def build_nc(qkv_dt=F32R, attn_dt=F32R, proj_dt=BF16):
    nc = bacc.Bacc(None)

    def dram_dt(dt):
        return F32 if dt == F32R else dt

    def cast(ap, dt):
        return ap.bitcast(F32R) if dt == F32R else ap

    # x^T stored block-major: [128, blk, kc, 512] so each 512-token block
    # is 16KB contiguous per partition.
    xt = nc.dram_tensor("xt", [128, NBLK, KC, 512], dram_dt(qkv_dt),
                        kind="ExternalInput")
    # weights pre-permuted to [128, kc, 128] (4KB contiguous/partition)
    wq = nc.dram_tensor("wq", [128, KC, 128], dram_dt(qkv_dt), kind="ExternalInput")
    wk = nc.dram_tensor("wk", [128, KC, 128], dram_dt(qkv_dt), kind="ExternalInput")
    wv = nc.dram_tensor("wv", [128, KC, 128], dram_dt(qkv_dt), kind="ExternalInput")
    wp = nc.dram_tensor("wp", [128, KC, 128], dram_dt(proj_dt), kind="ExternalInput")
    bq = nc.dram_tensor("bq", [128, 1], F32, kind="ExternalInput")
    bk = nc.dram_tensor("bk", [128, 1], F32, kind="ExternalInput")
    bv = nc.dram_tensor("bv", [128, 1], F32, kind="ExternalInput")
    bp = nc.dram_tensor("bp", [128, 1], F32, kind="ExternalInput")
    ident = nc.dram_tensor(
        "ident", [128, 128], dram_dt(attn_dt), kind="ExternalInput"
    )
    ones = nc.dram_tensor("ones", [128, 1], dram_dt(attn_dt), kind="ExternalInput")
    out = nc.dram_tensor("o", [128, TOK], F32, kind="ExternalOutput")

    # Collective staging (split per (batch, half) so early AllGathers
    # overlap later attention).
    ag_in = [
        [nc.dram_tensor(f"ag_in{b}_{h}", [128, HALF], proj_dt) for h in range(2)]
        for b in range(B)
    ]
    ag_out = [
        [
            nc.dram_tensor(f"ag_out{b}_{h}", [D, HALF], proj_dt,
                           addr_space="Shared")
            for h in range(2)
        ]
        for b in range(B)
    ]
    # split staging for the LAST chunk: one gather per head group, so the
    # first head's AllGather overlaps the second head's attention.
    ag_in3 = [nc.dram_tensor(f"ag_in3_{h}", [64, HALF], proj_dt)
              for h in range(2)]
    ag_out3 = [nc.dram_tensor(f"ag_out3_{h}", [D // 2, HALF], proj_dt,
                              addr_space="Shared") for h in range(2)]
    wp2a = nc.dram_tensor("wp2a", [128, KC // 2, 128], dram_dt(proj_dt),
                          kind="ExternalInput")
    wp2b = nc.dram_tensor("wp2b", [128, KC // 2, 128], dram_dt(proj_dt),
                          kind="ExternalInput")

    with tile.TileContext(nc) as tc, ExitStack() as ctx:
        wpool = ctx.enter_context(tc.tile_pool(name="w", bufs=1))
        qkpool = ctx.enter_context(tc.tile_pool(name="qk", bufs=1))
        vpool = ctx.enter_context(tc.tile_pool(name="v2", bufs=1))
        xpool = ctx.enter_context(tc.tile_pool(name="x", bufs=4))
        vtpool = ctx.enter_context(tc.tile_pool(name="vt", bufs=2))
        ptpool = ctx.enter_context(tc.tile_pool(name="pt", bufs=3))
        unpool = ctx.enter_context(tc.tile_pool(name="un", bufs=2))
        rpool = ctx.enter_context(tc.tile_pool(name="r", bufs=1))
        rdpool = ctx.enter_context(tc.tile_pool(name="rd", bufs=2, space="DRAM"))
        apool = ctx.enter_context(tc.tile_pool(name="ap", bufs=2))
        oupool = ctx.enter_context(tc.tile_pool(name="ou", bufs=2))
        # PSUM budget (8 banks of 2KB/partition):
        #   ps_one (bufs=1): po [65,1024]                -> 2 banks
        #   ps_mm (bufs=2): mmA [128,512] qkv/transpose/proj -> 2 banks
        #   ps_two (bufs=2): ss [128,1024]               -> 4 banks
        ps_one = ctx.enter_context(tc.tile_pool(name="ps1", bufs=1, space="PSUM"))
        ps_mm = ctx.enter_context(tc.tile_pool(name="psm", bufs=2, space="PSUM"))
        ps_two = ctx.enter_context(tc.tile_pool(name="ps2", bufs=2, space="PSUM"))

        # ---- weights / constants ----
        w_q = wpool.tile([128, KC, 128], qkv_dt, tag="w_q")
        w_k = wpool.tile([128, KC, 128], qkv_dt, tag="w_k")
        w_v = wpool.tile([128, KC, 128], qkv_dt, tag="w_v")
        w_p = wpool.tile([128, KC, 128], proj_dt, tag="w_p")
        w_p2a = wpool.tile([128, KC // 2, 128], proj_dt, tag="w_p2a")
        w_p2b = wpool.tile([128, KC // 2, 128], proj_dt, tag="w_p2b")
        b_q = wpool.tile([128, 1], F32, tag="b_q")
        b_k = wpool.tile([128, 1], F32, tag="b_k")
        b_v = wpool.tile([128, 1], F32, tag="b_v")
        b_p = wpool.tile([128, 1], F32, tag="b_p")
        id_t = wpool.tile([128, 128], attn_dt, tag="id_t")

        # qT/kT: [feature 128 (= 2 heads x 64), token 4096]; head hl in rows
        # hl*64:(hl+1)*64 so both S^T operands share a partition base.
        qT = qkpool.tile([128, TOK], attn_dt, tag="qT")
        kT = qkpool.tile([128, TOK], attn_dt, tag="kT")
        # V2: [token part, 32 token-chunks, 130]: v_h0 | ones | v_h1 | ones
        V2 = vpool.tile([128, TOK // 128, 130], attn_dt, tag="V2")

        # ---- startup DMA order: xb0 (gpsimd) + w_q (sync) in parallel
        # so the first QKV chain can start at ~20us.
        xbs = {}

        def issue_xb(blk):
            xb = xpool.tile([128, KC, 512], qkv_dt, tag="xb")
            eng = nc.sync if blk % 2 == 0 else nc.gpsimd
            eng.dma_start(out=xb, in_=cast(xt[:, blk], qkv_dt))
            xbs[blk] = xb

        nc.gpsimd.dma_start(out=b_q, in_=bq[:])
        xb0t = xpool.tile([128, KC, 512], qkv_dt, tag="xb")
        nc.gpsimd.dma_start(out=xb0t[:, 0:4], in_=cast(xt[:, 0, 0:4], qkv_dt))
        nc.sync.dma_start(out=w_q, in_=cast(wq[:], qkv_dt))
        nc.sync.dma_start(out=xb0t[:, 4:8], in_=cast(xt[:, 0, 4:8], qkv_dt))
        xbs[0] = xb0t
        nc.sync.dma_start(out=w_k, in_=cast(wk[:], qkv_dt))
        nc.sync.dma_start(out=w_v, in_=cast(wv[:], qkv_dt))
        for t, src in ((b_k, bk), (b_v, bv)):
            nc.gpsimd.dma_start(out=t, in_=src[:])
        nc.gpsimd.dma_start(out=id_t, in_=cast(ident[:], attn_dt))
        issue_xb(1)
        issue_xb(2)
        issue_xb(3)
        nc.gpsimd.dma_start(out=w_p, in_=cast(wp[:], proj_dt))
        nc.gpsimd.dma_start(out=w_p2a, in_=cast(wp2a[:], proj_dt))
        nc.gpsimd.dma_start(out=w_p2b, in_=cast(wp2b[:], proj_dt))
        nc.gpsimd.dma_start(out=b_p, in_=bp[:])
        # ones column of V2 via one tiny DMA + free-axis broadcast copies
        # (a broadcast DMA would generate 8192 4-byte descriptors).
        ones_r = wpool.tile([1, 128], attn_dt, tag="ones_r")
        nc.scalar.dma_start(
            out=ones_r, in_=cast(ones[:].rearrange("p o -> o p"), attn_dt)
        )
        nc.scalar.dma_start(
            out=V2[:, :, 64:65],
            in_=cast(ones[:].to_broadcast((128, TOK // 128, 1)), attn_dt),
        )
        nc.scalar.dma_start(
            out=V2[:, :, 129:130],
            in_=cast(ones[:].to_broadcast((128, TOK // 128, 1)), attn_dt),
        )

        # ---- work-unit emitters ----
        # Filler work (QKV chains, projection chains) is split into
        # micro-units of ~2 matmuls. Attention groups pull one unit
        # BETWEEN the S and PV matmuls of a step, so the PE has
        # independent work exactly where it would otherwise stall
        # waiting for ScalarE's exp — keeping the tensor engine
        # continuously busy (and at full DVFS clock).
        class Stream:
            def __init__(self):
                self.units = []

            def add(self, *units):
                self.units.extend(units)

            def pull(self):
                if self.units:
                    self.units.pop(0)()

            def drain(self):
                while self.units:
                    self.units.pop(0)()

        def qkv_chain_units(blk, name):
            """Micro-units for one q/k/v projection chain of a block."""
            w_t, b_t = {"q": (w_q, b_q), "k": (w_k, b_k), "v": (w_v, b_v)}[name]
            t0 = blk * 512
            state = {}

            def mk_mm(k0, k1):
                def f():
                    if "ps" not in state:
                        pst = ps_mm.tile([128, 512], F32, tag="mmA")
                        state["ps"] = pst
                    xb = xbs[blk]
                    for kc in range(k0, k1):
                        nc.tensor.matmul(
                            out=state["ps"], lhsT=w_t[:, kc, :],
                            rhs=xb[:, kc, :],
                            start=(kc == 0), stop=(kc == KC - 1),
                        )
                return f

            def fin():
                ps = state["ps"]
                if name == "q":
                    nc.vector.tensor_scalar_add(
                        out=qT[:, t0 : t0 + 512], in0=ps, scalar1=b_t
                    )
                elif name == "k":
                    nc.vector.tensor_scalar_add(
                        out=kT[:, t0 : t0 + 512], in0=ps, scalar1=b_t
                    )
                else:
                    vtmp = vtpool.tile([128, 512], attn_dt, tag="vtmp")
                    nc.vector.tensor_scalar_add(out=vtmp, in0=ps, scalar1=b_t)
                    state["vtmp"] = vtmp

            def mk_tr(sx):
                def f():
                    vtmp = state["vtmp"]
                    ch = blk * 4 + sx
                    ps_t = ps_mm.tile([128, 128], attn_dt, tag="mmA")
                    nc.tensor.transpose(
                        out=ps_t, in_=vtmp[:, sx * 128 : (sx + 1) * 128],
                        identity=id_t,
                    )
                    nc.vector.tensor_copy(out=V2[:, ch, 0:64], in_=ps_t[:, 0:64])
                    nc.vector.tensor_copy(
                        out=V2[:, ch, 65:129], in_=ps_t[:, 64:128]
                    )
                return f

            units = [mk_mm(0, 4), mk_mm(4, 8), fin]
            if name == "v":
                units += [mk_tr(sx) for sx in range(4)]
            return units

        def qkv_chain(blk, name):
            for u in qkv_chain_units(blk, name):
                u()

        abs_ = {}

        def issue_ab(b, hf):
            ag_r = ag_out[b][hf].rearrange("(kc p) t -> p kc t", p=128)
            ab = apool.tile([128, KC, HALF], proj_dt, tag="ab")
            nc.gpsimd.dma_start(out=ab, in_=ag_r)
            abs_[(b, hf)] = ab

        def proj_chain_units(b, hf, i2):
            i0 = i2 * 512
            state = {}

            def mk_mm(k0, k1):
                def f():
                    if "ps" not in state:
                        pst = ps_mm.tile([128, 512], F32, tag="mmA")
                        state["ps"] = pst
                    ab = abs_[(b, hf)]
                    for kc in range(k0, k1):
                        nc.tensor.matmul(
                            out=state["ps"], lhsT=w_p[:, kc, :],
                            rhs=ab[:, kc, i0 : i0 + 512],
                            start=(kc == 0), stop=(kc == KC - 1),
                        )
                return f

            def fin():
                ot = oupool.tile([128, 512], F32, tag="ot")
                nc.vector.tensor_scalar_add(out=ot, in0=state["ps"], scalar1=b_p)
                to = b * N + hf * HALF + i0
                nc.sync.dma_start(out=out[:, to : to + 512], in_=ot)

            return [mk_mm(0, 4), mk_mm(4, 8), fin]

        def proj_chain(b, hf, i2):
            for u in proj_chain_units(b, hf, i2):
                u()

        def attn_group(b, ib, hl, stream, pull_every=1, fast_stage=False,
                       pull2_until=0):
            """One (batch, query-block, head) attention group. Pulls one
            stream unit between the S and PV matmuls of each step."""
            hs = hl * 64
            voff = hl * 65
            i0 = b * N + ib * IBLK
            ps_o = ps_one.tile([65, IBLK], F32, tag="po")

            def s_step(jc):
                j0 = b * N + jc * 128
                ps_s = ps_two.tile([128, IBLK], F32, tag="ss")
                for su in range(NSUB):
                    nc.tensor.matmul(
                        out=ps_s[:, su * 512 : (su + 1) * 512],
                        lhsT=kT[hs : hs + 64, j0 : j0 + 128],
                        rhs=qT[hs : hs + 64, i0 + su * 512 : i0 + (su + 1) * 512],
                        start=True, stop=True,
                    )
                pt = ptpool.tile([128, IBLK], attn_dt, tag="pt")
                nc.scalar.activation(out=pt, in_=ps_s, func=EXP, scale=float(SCALE))
                return pt

            pt_q = [s_step(0), s_step(1)]
            for jc in range(NJC):
                pt_cur = pt_q.pop(0)
                if jc + 2 < NJC:
                    pt_q.append(s_step(jc + 2))
                if jc % pull_every == 0:
                    stream.pull()
                    if jc < pull2_until:
                        stream.pull()
                for su in range(NSUB):
                    nc.tensor.matmul(
                        out=ps_o[:, su * 512 : (su + 1) * 512],
                        lhsT=V2[:, ((b * N + jc * 128) // 128), voff : voff + 65],
                        rhs=pt_cur[:, su * 512 : (su + 1) * 512],
                        start=(jc == 0), stop=(jc == NJC - 1),
                    )
            # single copy (rows + denominator) releases PSUM.
            und = unpool.tile([65, IBLK], F32, tag="und")
            nc.vector.tensor_copy(out=und, in_=ps_o)
            unr = unpool.tile([64, IBLK], proj_dt, tag="unr")
            if fast_stage:
                # recip of the whole tile (same DVE cost as one row), then
                # replicate the denominator row across 64 partitions with
                # a rank-1 matmul: ~8us from last PV to the AG trigger.
                rc65 = rpool.tile([65, IBLK], F32, tag="rc65")
                nc.vector.reciprocal_approx_fast(out=rc65, in_=und)
                rcr = rpool.tile([1, IBLK], attn_dt, tag="rcr")
                nc.vector.tensor_copy(out=rcr, in_=rc65[64:65, :])
                for su in range(NSUB):
                    ps_bc = ps_mm.tile([64, 512], F32, tag="mmA")
                    nc.tensor.matmul(
                        out=ps_bc,
                        lhsT=ones_r[:, 0:64],
                        rhs=rcr[:, su * 512 : (su + 1) * 512],
                        start=True, stop=True,
                    )
                    nc.vector.tensor_mul(
                        out=unr[:, su * 512 : (su + 1) * 512],
                        in0=und[0:64, su * 512 : (su + 1) * 512],
                        in1=ps_bc,
                    )
                nc.sync.dma_start(out=ag_in3[hl][:], in_=unr)
                nc.gpsimd.collective_compute(
                    "AllGather", mybir.AluOpType.bypass,
                    ins=[ag_in3[hl][:]], outs=[ag_out3[hl][:]],
                    replica_groups=[list(range(NCORES))],
                )
            else:
                rd = rdpool.tile([1, IBLK], F32, tag="rd")
                nc.gpsimd.dma_start(out=rd, in_=und[64:65, :])
                rr = rpool.tile([64, IBLK], F32, tag="rr")
                nc.gpsimd.dma_start(out=rr, in_=rd.to_broadcast((64, IBLK)))
                rcp = rpool.tile([64, IBLK], F32, tag="rcp")
                nc.vector.reciprocal_approx_fast(out=rcp, in_=rr)
                nc.gpsimd.tensor_mul(out=unr, in0=und[0:64, :], in1=rcp)
                nc.gpsimd.dma_start(out=ag_in[b][ib][hs : hs + 64, :], in_=unr)

        def attn_chunk(b, ib, stream, pull_every=1, fast_stage=False,
                       pull2_until=0):
            attn_group(b, ib, 0, stream, pull_every, fast_stage, pull2_until)
            attn_group(b, ib, 1, stream, pull_every, fast_stage)
            if not fast_stage:
                nc.gpsimd.collective_compute(
                    "AllGather", mybir.AluOpType.bypass,
                    ins=[ag_in[b][ib][:]], outs=[ag_out[b][ib][:]],
                    replica_groups=[list(range(NCORES))],
                )

        # ================= schedule =================
        # Phase A0: QKV for blocks 0-1 only (first attention group needs
        # qT tokens 0-1023 + the first kT/V2 chunks); blocks 2-3 stream
        # into the first chunk at 2 units/step (k2 lands by step ~2,
        # v2 by ~5, k3 by ~7, v3 by ~10 — all before their consumers).
        for blk in (0, 1):
            for name in ("q", "k", "v"):
                qkv_chain(blk, name)

        s0 = Stream()
        for blk in (2, 3):
            s0.add(*qkv_chain_units(blk, "k"))
            s0.add(*qkv_chain_units(blk, "v"))
        s0.add(*qkv_chain_units(2, "q"))
        s0.add(*qkv_chain_units(3, "q"))
        s0.add(lambda: issue_xb(4), lambda: issue_xb(5))
        for blk in (4, 5):
            s0.add(*qkv_chain_units(blk, "k"))
        s0.add(lambda: issue_xb(6), lambda: issue_xb(7))
        for blk in (6, 7):
            s0.add(*qkv_chain_units(blk, "k"))
        for blk in range(4, 8):
            s0.add(*qkv_chain_units(blk, "q"))
        for blk in range(4, 8):
            s0.add(*qkv_chain_units(blk, "v"))
        attn_chunk(0, 0, s0, pull2_until=14)
        attn_chunk(0, 1, s0)
        s0.drain()

        # b1 attention streams the batch-0 projection.
        s1 = Stream()
        s1.add(lambda: issue_ab(0, 0))
        s1.add(*proj_chain_units(0, 0, 0))
        s1.add(*proj_chain_units(0, 0, 1))
        s1.add(lambda: issue_ab(0, 1))
        s1.add(*proj_chain_units(0, 1, 0))
        s1.add(*proj_chain_units(0, 1, 1))
        attn_chunk(1, 0, s1, pull_every=1)
        s1.add(lambda: issue_ab(1, 0))
        attn_chunk(1, 1, s1, pull_every=1, fast_stage=True)
        s1.drain()

        # tail: batch-1 projection. The last chunk was gathered per-head,
        # so its projection contracts 4 kc chunks from each gathered half
        # (w_p2a/w_p2b rows are permuted to match).
        proj_chain(1, 0, 0)
        proj_chain(1, 0, 1)
        ab3 = []
        for h in range(2):
            ag3r = ag_out3[h].rearrange("(kc p) t -> p kc t", p=128)
            a3 = apool.tile([128, KC // 2, HALF], proj_dt, tag=f"ab3_{h}")
            nc.gpsimd.dma_start(out=a3, in_=ag3r)
            ab3.append(a3)
        for i2 in range(2):
            i0 = i2 * 512
            ps = ps_mm.tile([128, 512], F32, tag="mmA")
            for kc in range(KC // 2):
                nc.tensor.matmul(
                    out=ps, lhsT=w_p2a[:, kc, :], rhs=ab3[0][:, kc, i0 : i0 + 512],
                    start=(kc == 0), stop=False,
                )
            for kc in range(KC // 2):
                nc.tensor.matmul(
                    out=ps, lhsT=w_p2b[:, kc, :], rhs=ab3[1][:, kc, i0 : i0 + 512],
                    start=False, stop=(kc == KC // 2 - 1),
                )
            ot = oupool.tile([128, 512], F32, tag="ot")
            nc.vector.tensor_scalar_add(out=ot, in0=ps, scalar1=b_p)
            to = N + HALF + i0
            nc.sync.dma_start(out=out[:, to : to + 512], in_=ot)

    nc.compile()
    return nc


def np_dt(dt):
    return mybir.dt.np(F32 if dt == F32R else dt)


def prep_in_maps(x, Wqkv, bqkv, Wproj, bproj, qkv_dt=F32R, attn_dt=F32R,
                 proj_dt=BF16):
    x = np.asarray(x, dtype=np.float32)
    Wqkv = np.asarray(Wqkv, dtype=np.float32)
    bqkv = np.asarray(bqkv, dtype=np.float32)
    Wproj = np.asarray(Wproj, dtype=np.float32)
    bproj = np.asarray(bproj, dtype=np.float32)

    # x^T block-major: [128, blk, kc, 512]; row kc*128+p of x^T.
    xT = np.ascontiguousarray(x.reshape(TOK, D).T)     # [D, TOK]
    xtn = np.ascontiguousarray(
        xT.reshape(KC, 128, NBLK, 512).transpose(1, 2, 0, 3)
    ).astype(np_dt(qkv_dt))
    identity = np.eye(128, dtype=np_dt(attn_dt))
    ones_col = np.ones((128, 1), dtype=np_dt(attn_dt))

    def perm_w(w):  # [D', 128] -> [128, D'//128, 128] with row kc*128+p
        return np.ascontiguousarray(
            w.reshape(-1, 128, w.shape[1]).transpose(1, 0, 2)
        )

    # AllGather output rows are rank-major: row c*128 + hl*64 + d holds
    # feature (2c+hl)*64 + d; permute Wproj's contraction rows to match.
    wp_row_perm = np.empty(D, dtype=np.int64)
    for cc in range(NCORES):
        for hlhl in range(2):
            rows = np.arange(64)
            wp_row_perm[cc * 128 + hlhl * 64 + rows] = (2 * cc + hlhl) * 64 + rows

    # Split-gather permutations: gathered half h row r = c*64 + d holds
    # feature (2c+h)*64 + d; contraction chunk j covers cores 2j, 2j+1.
    wp2_perm = [np.empty(D // 2, dtype=np.int64) for _ in range(2)]
    for hh in range(2):
        for cc in range(NCORES):
            rows = np.arange(64)
            wp2_perm[hh][cc * 64 + rows] = (2 * cc + hh) * 64 + rows

    # qkv column index for (head h, depth d, which): h*192 + d*3 + which
    d_idx = np.arange(DEPTH)
    in_maps = []
    for c in range(NCORES):
        h0, h1 = 2 * c, 2 * c + 1
        qcols = np.concatenate([h0 * 192 + 3 * d_idx, h1 * 192 + 3 * d_idx])
        kcols = qcols + 1
        vcols = qcols + 2
        in_maps.append(
            {
                "xt": xtn,
                "wq": perm_w(Wqkv[:, qcols]).astype(np_dt(qkv_dt)),
                "wk": perm_w(Wqkv[:, kcols]).astype(np_dt(qkv_dt)),
                "wv": perm_w(Wqkv[:, vcols]).astype(np_dt(qkv_dt)),
                "wp": perm_w(
                    Wproj[wp_row_perm, 128 * c : 128 * (c + 1)]
                ).astype(np_dt(proj_dt)),
                "wp2a": perm_w(
                    Wproj[wp2_perm[0], 128 * c : 128 * (c + 1)]
                ).astype(np_dt(proj_dt)),
                "wp2b": perm_w(
                    Wproj[wp2_perm[1], 128 * c : 128 * (c + 1)]
                ).astype(np_dt(proj_dt)),
                "bq": np.ascontiguousarray(bqkv[qcols]).reshape(128, 1),
                "bk": np.ascontiguousarray(bqkv[kcols]).reshape(128, 1),
                "bv": np.ascontiguousarray(bqkv[vcols]).reshape(128, 1),
                "bp": np.ascontiguousarray(
                    bproj[128 * c : 128 * (c + 1)]
                ).reshape(128, 1),
                "ident": identity,
                "ones": ones_col,
            }
        )
    return in_maps


def assemble(results):
    outT = np.concatenate([r["o"] for r in results], axis=0)  # [D, TOK]
    return np.ascontiguousarray(outT.T).reshape(B, N, D).astype(np.float32)


CONFIG = {"qkv_dt": BF16, "attn_dt": F32R, "proj_dt": BF16}

_NC_CACHE = {}


def get_nc():
    if "nc" not in _NC_CACHE:
        _NC_CACHE["nc"] = build_nc(**CONFIG)
    return _NC_CACHE["nc"]


def kernel(x, Wqkv, bqkv, Wproj, bproj):
    nc = get_nc()
    in_maps = prep_in_maps(x, Wqkv, bqkv, Wproj, bproj, **CONFIG)
    res = run_bass_kernel_spmd(nc, in_maps, list(range(NCORES)))
    return assemble(res.results)
